# revision 24
# baseline (speedup 1.0000x reference)
"""Trainium2 Bass kernel for nn_AIModel_34892314312864 (ragged_sequence).

Model (per sample):
  pools  = gather-sum embeddings (diagnose[40], procedures[30], masked last_meds[20])
  logits = concat(pools) @ W_out.T + b_out
  hist   = last hidden of a length-masked GRU over med_hist [T=8, V=512]
  out    = logits + hist
  score  = 0.0005 * mean_b( sigmoid(out)_b @ ddi_adj @ sigmoid(out)_b )

Distribution: pure data parallel, batch 512 sharded 64/core across 8 cores.
Weights replicated. DDI score: per-core partial sums, combined on host.

Key device techniques:
  - Embedding pools: one indirect-DMA gather per table ([64, n*64] f32,
    sample-major) + one strided DVE reduce.
  - All PE compute in bf16. Weights W_ih/W_hh/W_out are transposed to
    v-major during load with zero extra traffic: the f32 HBM tensor is
    viewed as uint16, the odd (high) halves are gathered via the HWDGE
    xbar-transpose DMA; a f32 high-half IS the bf16 truncation.
  - GRU: gi = x @ W_ih.T precomputed for all steps (M=128 matmuls);
    recurrent h @ W_hh.T per step with h kept both sample-major (gate
    math) and v-major (matmul lhsT, via PE transposes).
"""

import sys

sys.path.insert(0, "/opt/trn_rl_repo")

import numpy as np

from concourse import bass, library_config, mybir
from concourse.tile import TileContext

F32 = mybir.dt.float32
BF16 = mybir.dt.bfloat16
I32 = mybir.dt.int32
I16 = mybir.dt.int16
U16 = mybir.dt.uint16
AF = mybir.ActivationFunctionType
ALU = mybir.AluOpType

# Problem constants (hardcoded; kernel.py must be self-contained)
B, D, P, M, T, V, E = 512, 40, 30, 20, 8, 512, 64
DV, PV = 2000, 1500
MED_PAD = V + 2  # 514
G = 3 * V  # 1536
N_CORES = 8
BL = B // N_CORES  # 64 samples per core
KGLOSS_SCALE = 0.001 * 0.5

DIAG_ROWS, PROC_ROWS, MED_ROWS = DV + 3, PV + 3, V + 3


def _u16_hi_cols(ap_f32, col0, col1):
    """View f32 DRAM AP as uint16 and select the high halves of f32
    columns [col0, col1) -> shape [rows, col1-col0] u16 (strided)."""
    u = ap_f32.bitcast(U16)
    return u[:, 2 * col0 + 1 : 2 * col1 : 2]


def build(use_biases: bool, debug: bool = False):
    nc = bass.Bass()
    dbg_ext = {}
    if debug:
        for nm, shape in [
            ("dbg_fin", [BL, 3 * E]),
            ("dbg_gi0", [128, G]),
            ("dbg_h0", [BL, V]),
            ("dbg_h1", [BL, V]),
            ("dbg_hF", [BL, V]),
            ("dbg_wi0", [128, G]),
            ("dbg_xt0", [128, 512]),
        ]:
            dbg_ext[nm] = nc.declare_dram_parameter(nm, shape, F32, isOutput=True)

    # ---- parameters (per-core shards / replicated weights) ----
    diag_idx16 = nc.declare_dram_parameter("diag_idx16", [16, BL * D // 16], I16, isOutput=False)
    proc_idx16 = nc.declare_dram_parameter("proc_idx16", [16, BL * P // 16], I16, isOutput=False)
    med_idx16 = nc.declare_dram_parameter("med_idx16", [16, BL * M // 16], I16, isOutput=False)
    last_meds = nc.declare_dram_parameter("last_meds", [BL, M], I32, isOutput=False)
    med_hist = nc.declare_dram_parameter("med_hist", [BL, T, V], F32, isOutput=False)
    hist_len = nc.declare_dram_parameter("hist_len", [BL, 1], I32, isOutput=False)
    ddi_adj = nc.declare_dram_parameter("ddi_adj", [V, V], F32, isOutput=False)
    diag_table = nc.declare_dram_parameter("diag_table", [DIAG_ROWS, E], F32, isOutput=False)
    proc_table = nc.declare_dram_parameter("proc_table", [PROC_ROWS, E], F32, isOutput=False)
    med_table = nc.declare_dram_parameter("med_table", [MED_ROWS, E], F32, isOutput=False)
    # W_out is zero-padded on host from [V, 192] to [V, 256] so its transpose
    # splits into two full 128-row chunks
    W_out = nc.declare_dram_parameter("W_out", [V, 4 * E], F32, isOutput=False)
    b_out = nc.declare_dram_parameter("b_out", [1, V], F32, isOutput=False)
    W_ih = nc.declare_dram_parameter("W_ih", [G, V], F32, isOutput=False)
    W_hh = nc.declare_dram_parameter("W_hh", [G, V], F32, isOutput=False)
    b_ih = nc.declare_dram_parameter("b_ih", [1, G], F32, isOutput=False)
    b_hh = nc.declare_dram_parameter("b_hh", [1, G], F32, isOutput=False)
    out_ext = nc.declare_dram_parameter("out", [BL, V], F32, isOutput=True)
    ddi_ext = nc.declare_dram_parameter("ddi", [1, 1], F32, isOutput=True)

    # ---- inline constants ----
    import ml_dtypes

    ident_np = np.zeros((128, 128), np.float32)
    np.fill_diagonal(ident_np, 1.0)

    ident_c = nc.inline_tensor(ident_np.astype(ml_dtypes.bfloat16), name="ident_bf")
    iota_c = nc.inline_tensor(
        np.tile(np.arange(T, dtype=np.float32), (BL, 1)), name="iota8"
    )
    ones_col_c = nc.inline_tensor(np.ones((BL, 1), np.float32), name="ones_col")
    # iota2c[p, c] = 2c + (p >= 64): the slot number at (partition, chunk)
    iota2c_np = (2 * np.arange(M // 2, dtype=np.float32))[None, :] + (
        np.arange(128) >= BL
    ).astype(np.float32)[:, None]
    iota2c_c = nc.inline_tensor(iota2c_np, name="iota2c")
    # pairsel[p, b] = 1 if p % 64 == b (sums partition b and 64+b)
    pairsel_np = np.zeros((128, BL), np.float32)
    pairsel_np[np.arange(128), np.arange(128) % BL] = 1.0
    pairsel_c = nc.inline_tensor(pairsel_np.astype(ml_dtypes.bfloat16), name="pairsel")
    if use_biases:
        ones_row_c = nc.inline_tensor(
            np.ones((1, 128), ml_dtypes.bfloat16), name="ones_row"
        )
        # mask that keeps r,z parts of b_hh and zeroes the n part
        rz_np = np.zeros((1, G), np.float32)
        rz_np[:, : 2 * V] = 1.0
        rz_mask_c = nc.inline_tensor(rz_np, name="rz_mask")

    with TileContext(nc) as tc:
        with (
            tc.tile_pool(name="wts", bufs=1) as wp,
            tc.tile_pool(name="work", bufs=2) as kp,
            tc.tile_pool(name="gath", bufs=1) as gp,
            tc.tile_pool(name="psum", bufs=3, space="PSUM") as pgp,
            tc.tile_pool(name="psum_h", bufs=3, space="PSUM") as php,
            tc.tile_pool(name="psum_t", bufs=2, space="PSUM") as ptp,
        ):
            # PSUM budget (8 banks): pgp tag "mm" [128,512] x3 + php tag
            # "ph" [64,512] x3 + ptp tag "pt" [128,128] x2 = 8.
            nc.gpsimd.load_library(library_config.mlp)
            # ======== constants to SBUF ========
            ident = wp.tile([128, 128], BF16, tag="ident")
            nc.scalar.dma_start(out=ident[:], in_=ident_c[:])
            iota = wp.tile([BL, T], F32, tag="iota")
            nc.scalar.dma_start(out=iota[:], in_=iota_c[:])
            ones_col = wp.tile([BL, 1], F32, tag="ones_col")
            nc.scalar.dma_start(out=ones_col[:], in_=ones_col_c[:])
            iota2c = wp.tile([128, M // 2], F32, tag="iota2c")
            nc.scalar.dma_start(out=iota2c[:], in_=iota2c_c[:])
            pairsel = wp.tile([128, BL], BF16, tag="pairsel")
            nc.scalar.dma_start(out=pairsel[:], in_=pairsel_c[:])

            # ======== weight loads: cast to bf16 in DRAM, xbar-transpose in ====
            # one SWDGE cast-DMA per tensor (f32 HBM -> bf16 DRAM scratch),
            # then HWDGE xbar transposes DRAM -> SBUF v-major chunks.
            wih_bf = nc.dram_tensor("wih_bf", [G, V], BF16)
            whh_bf = nc.dram_tensor("whh_bf", [G, V], BF16)
            wout_bf = nc.dram_tensor("wout_bf", [V, 4 * E], BF16)
            x_bf = nc.dram_tensor("x_bf", [T * BL, V], BF16)
            nc.gpsimd.dma_start(out=whh_bf[:], in_=W_hh[:])
            nc.gpsimd.dma_start(out=x_bf[:], in_=med_hist[:].rearrange("b t v -> t b v"))
            nc.gpsimd.dma_start(out=wih_bf[:], in_=W_ih[:])
            nc.gpsimd.dma_start(out=wout_bf[:], in_=W_out[:])

            wi = [wp.tile([128, G], BF16, tag=f"wi{c}", name=f"wi{c}") for c in range(4)]
            wh = [wp.tile([128, G], BF16, tag=f"wh{c}", name=f"wh{c}") for c in range(4)]
            xt = [wp.tile([128, T * BL], BF16, tag=f"xt{c}", name=f"xt{c}") for c in range(4)]
            wo0 = wp.tile([128, V], BF16, tag="wo0")
            wo1 = wp.tile([128, V], BF16, tag="wo1")
            for c in range(4):
                nc.sync.dma_start(
                    out=wh[c][:], in_=whh_bf[:, 128 * c : 128 * (c + 1)], transpose=True
                )
            for c in range(4):
                nc.sync.dma_start(
                    out=xt[c][:], in_=x_bf[:, 128 * c : 128 * (c + 1)], transpose=True
                )
            for c in range(4):
                nc.sync.dma_start(
                    out=wi[c][:], in_=wih_bf[:, 128 * c : 128 * (c + 1)], transpose=True
                )
            nc.sync.dma_start(out=wo0[:], in_=wout_bf[:, 0:128], transpose=True)
            nc.sync.dma_start(out=wo1[:], in_=wout_bf[:, 128:256], transpose=True)

            # ======== embedding gathers + pools (dma_gather) ========
            # Index lists are host-prepped int16 in dma_gather's wrapped
            # layout with flat order i = b + 64*s, so gathered row i lands at
            # partition (b + 64*(s%2)), chunk s//2: sample b's even slots sit
            # in partition b, odd slots in partition 64+b, slots along the
            # free dim. Pool = free-dim reduce + one pair-sum matmul.
            psum_fin = pgp.tile([BL, 3 * E], F32, tag="mm", name="psum_fin")

            def emit_pool(idx16_dram, n_slots, table_dram, out_col, mask=None):
                n_idx = BL * n_slots
                nch = n_slots // 2
                it = gp.tile(
                    [128, n_idx // 16], I32, tag=f"it{out_col}", name=f"it{out_col}"
                )
                it16 = it[:].bitcast(I16)  # [128, n_idx/8]; use cols [0, n_idx/16)
                # each of the 8 Q7 cores reads its own 16-partition copy
                for r in range(8):
                    nc.scalar.dma_start(
                        out=it16[16 * r : 16 * (r + 1), 0 : n_idx // 16],
                        in_=idx16_dram[:],
                    )
                g = gp.tile([128, nch * E], F32, tag=f"g{out_col}", name=f"g{out_col}")
                nc.gpsimd.dma_gather(
                    out_ap=g[:].rearrange("p (c f) -> p c f", c=nch, f=E),
                    in_ap=table_dram,
                    idxs_ap=it16[:, 0 : n_idx // 16],
                    num_idxs=n_idx,
                    num_idxs_reg=n_idx,
                    elem_size=E,
                    single_packet=False,
                )
                g3 = g[:].rearrange("p (c f) -> p f c", c=nch, f=E)
                if mask is not None:
                    nc.vector.tensor_tensor(
                        out=g3,
                        in0=g3,
                        in1=mask[:]
                        .rearrange("p (o c) -> p o c", o=1)
                        .to_broadcast([128, E, nch]),
                        op=ALU.mult,
                    )
                part = gp.tile([128, E], F32, tag=f"part{out_col}", name=f"part{out_col}")
                nc.vector.tensor_reduce(
                    out=part[:], in_=g3, axis=mybir.AxisListType.X, op=ALU.add
                )
                part_bf = gp.tile(
                    [128, E], BF16, tag=f"partbf{out_col}", name=f"partbf{out_col}"
                )
                nc.vector.tensor_copy(out=part_bf[:], in_=part[:])
                nc.tensor.matmul(
                    out=psum_fin[:, out_col * E : (out_col + 1) * E],
                    lhsT=pairsel[:],
                    rhs=part_bf[:],
                    start=True,
                    stop=True,
                )

            emit_pool(diag_idx16[:], D, diag_table[:], 0)
            emit_pool(proc_idx16[:], P, proc_table[:], 1)

            # med mask: keep[b] = #non-pad ids; mask[p, c] = (2c + (p>=64)) < keep
            med_i = gp.tile([BL, M], I32, tag="med_i")
            nc.scalar.dma_start(out=med_i[:], in_=last_meds[:])
            med_f = gp.tile([BL, M], F32, tag="med_f")
            nc.vector.tensor_copy(out=med_f[:], in_=med_i[:])
            nped = gp.tile([BL, M], F32, tag="nped")
            nc.vector.tensor_scalar(
                out=nped[:], in0=med_f[:], scalar1=float(MED_PAD), scalar2=None,
                op0=ALU.not_equal,
            )
            keep = gp.tile([BL, 1], F32, tag="keep")
            nc.vector.tensor_reduce(
                out=keep[:], in_=nped[:], axis=mybir.AxisListType.X, op=ALU.add
            )
            keep2 = gp.tile([128, 1], F32, tag="keep2")
            nc.vector.tensor_copy(out=keep2[0:BL, :], in_=keep[:])
            nc.sync.dma_start(out=keep2[BL:128, :], in_=keep[:])
            med_msk = gp.tile([128, M // 2], F32, tag="med_msk")
            nc.vector.tensor_scalar(
                out=med_msk[:], in0=iota2c[:], scalar1=keep2[:], scalar2=None,
                op0=ALU.is_lt,
            )
            emit_pool(med_idx16[:], M, med_table[:], 2, mask=med_msk)

            fin = wp.tile([BL, 3 * E], F32, tag="fin")
            nc.vector.tensor_copy(out=fin[:], in_=psum_fin[:])

            # fin -> bf16, then fin.T chunks for the logits matmul
            fin_bf = wp.tile([BL, 4 * E], BF16, tag="fin_bf")
            nc.vector.memset(fin_bf[:, 3 * E : 4 * E], 0.0)
            nc.vector.tensor_copy(out=fin_bf[:, 0 : 3 * E], in_=fin[:])
            if debug:
                nc.gpsimd.dma_start(out=dbg_ext["dbg_fin"][:], in_=fin[:])
            finT0 = wp.tile([128, BL], BF16, tag="finT0")
            finT1 = wp.tile([128, BL], BF16, tag="finT1")
            pt0 = ptp.tile([128, BL], BF16, tag="pt")
            nc.tensor.transpose(out=pt0[:], in_=fin_bf[:, 0:128], identity=ident[:BL, :BL])
            nc.vector.tensor_copy(out=finT0[:], in_=pt0[:])
            pt1 = ptp.tile([128, BL], BF16, tag="pt")
            nc.tensor.transpose(
                out=pt1[:], in_=fin_bf[:, 128:256], identity=ident[:BL, :BL]
            )
            nc.vector.tensor_copy(out=finT1[:], in_=pt1[:])

            # ======== hist-len mask ========
            hl_i = gp.tile([BL, 1], I32, tag="hl_i")
            nc.scalar.dma_start(out=hl_i[:], in_=hist_len[:])
            hl_f = gp.tile([BL, 1], F32, tag="hl_f")
            nc.vector.tensor_copy(out=hl_f[:], in_=hl_i[:])
            tmask = wp.tile([BL, T], F32, tag="tmask")
            nc.vector.tensor_scalar(
                out=tmask[:], in0=iota[:], scalar1=hl_f[:], scalar2=None, op0=ALU.is_lt
            )

            # ======== ddi_adj -> bf16 (one cast-DMA) ========
            A_bf = wp.tile([128, 4 * V], BF16, tag="abf")
            nc.gpsimd.dma_start(
                out=A_bf[:], in_=ddi_adj[:].rearrange("(k p) v -> p k v", p=128)
            )

            # ======== biases ========
            if use_biases:
                ones_row = wp.tile([1, 128], BF16, tag="ones_row")
                nc.scalar.dma_start(out=ones_row[:], in_=ones_row_c[:])
                bo_bf = wp.tile([1, V], BF16, tag="bo_bf")
                nc.gpsimd.dma_start(out=bo_bf[:], in_=b_out[:])
                # gi bias = b_ih + rz_mask * b_hh ; n-part of b_hh added per step
                bi_f = wp.tile([1, G], F32, tag="bi_f")
                nc.scalar.dma_start(out=bi_f[:], in_=b_ih[:])
                bh_f = wp.tile([1, G], F32, tag="bh_f")
                nc.scalar.dma_start(out=bh_f[:], in_=b_hh[:])
                rz_m = wp.tile([1, G], F32, tag="rz_m")
                nc.scalar.dma_start(out=rz_m[:], in_=rz_mask_c[:])
                gi_bias = wp.tile([1, G], F32, tag="gi_bias")
                nc.vector.tensor_tensor(
                    out=gi_bias[:], in0=bh_f[:], in1=rz_m[:], op=ALU.mult
                )
                nc.vector.tensor_tensor(
                    out=gi_bias[:], in0=gi_bias[:], in1=bi_f[:], op=ALU.add
                )
                gi_bias_bf = wp.tile([1, G], BF16, tag="gi_bias_bf")
                nc.vector.tensor_copy(out=gi_bias_bf[:], in_=gi_bias[:])
                bh_n_bf = wp.tile([1, V], BF16, tag="bh_n_bf")
                nc.vector.tensor_copy(out=bh_n_bf[:], in_=bh_f[:, 2 * V : 3 * V])
                # materialize b_hh_n broadcast to all samples (K=1 matmul)
                pbh = pgp.tile([BL, V], F32, tag="mm")
                nc.tensor.matmul(
                    out=pbh[:], lhsT=ones_row[:, :BL], rhs=bh_n_bf[:],
                    start=True, stop=True,
                )
                bhn_mat = wp.tile([BL, V], BF16, tag="bhn_mat")
                nc.vector.tensor_copy(out=bhn_mat[:], in_=pbh[:])

            # ======== gi = x @ W_ih.T (+ folded biases), all 8 steps ========
            gi_sb = [wp.tile([128, G], BF16, tag=f"gi{j}", name=f"gi{j}") for j in range(4)]
            # odd steps live at partition base 64 inside gi_sb; compute
            # engines need base-0 operands, so DMA the odd halves down
            gi_odd = [
                wp.tile([BL, G], BF16, tag=f"giodd{j}", name=f"giodd{j}")
                for j in range(4)
            ]

            def emit_gi_tile(j):
                for ns in range(3):
                    pg = pgp.tile([128, 512], F32, tag="mm")
                    for c in range(4):
                        nc.tensor.matmul(
                            out=pg[:],
                            lhsT=xt[c][:, 128 * j : 128 * (j + 1)],
                            rhs=wi[c][:, 512 * ns : 512 * (ns + 1)],
                            start=(c == 0),
                            stop=(c == 3) if not use_biases else False,
                        )
                    if use_biases:
                        nc.tensor.matmul(
                            out=pg[:],
                            lhsT=ones_row[:, :128],
                            rhs=gi_bias_bf[:, 512 * ns : 512 * (ns + 1)],
                            start=False,
                            stop=True,
                        )
                    nc.vector.tensor_copy(
                        out=gi_sb[j][:, 512 * ns : 512 * (ns + 1)], in_=pg[:]
                    )
                nc.sync.dma_start(out=gi_odd[j][:], in_=gi_sb[j][BL:128, :])

            # ======== GRU recurrence ========
            h_sb = wp.tile([BL, V], BF16, tag="h_sb")
            hT = wp.tile([128, 4 * BL], BF16, tag="hT")

            def gate_slices(t):
                """gi slices for step t: [64, 512] each of r, z, n (base 0)."""
                j = t // 2
                sl = gi_sb[j][0:BL, :] if t % 2 == 0 else gi_odd[j][:]
                return sl[:, 0:V], sl[:, V : 2 * V], sl[:, 2 * V : 3 * V]

            def emit_hT(t):
                ptt = ptp.tile([128, 4 * BL], BF16, tag="pt", name=f"ptt{t}")
                for c in range(4):
                    nc.tensor.transpose(
                        out=ptt[:, BL * c : BL * (c + 1)],
                        in_=h_sb[:, 128 * c : 128 * (c + 1)],
                        identity=ident[:BL, :BL],
                    )
                nc.vector.tensor_copy(out=hT[:], in_=ptt[:])

            def emit_step0():
                gi_r, gi_z, gi_n = gate_slices(0)
                zp = kp.tile([BL, V], BF16, tag="zp")
                nc.scalar.activation(out=zp[:], in_=gi_z, func=AF.Sigmoid, scale=-1.0)
                n_t = kp.tile([BL, V], BF16, tag="n_t")
                if use_biases:
                    # n = tanh(gi_n + sigmoid(gi_r) * b_hh_n)   (h0 = 0)
                    r_t = kp.tile([BL, V], BF16, tag="r_t")
                    nc.scalar.activation(out=r_t[:], in_=gi_r, func=AF.Sigmoid)
                    p_t = kp.tile([BL, V], BF16, tag="p_t")
                    nc.vector.tensor_tensor(
                        out=p_t[:], in0=r_t[:], in1=bhn_mat[:], op=ALU.mult
                    )
                    a_n = kp.tile([BL, V], BF16, tag="a_n")
                    nc.vector.tensor_tensor(out=a_n[:], in0=p_t[:], in1=gi_n, op=ALU.add)
                    nc.scalar.activation(out=n_t[:], in_=a_n[:], func=AF.Tanh)
                else:
                    nc.scalar.activation(out=n_t[:], in_=gi_n, func=AF.Tanh)
                # h = m * z' * n
                w_t = kp.tile([BL, V], BF16, tag="w_t")
                nc.vector.tensor_scalar(
                    out=w_t[:], in0=zp[:], scalar1=tmask[:, 0:1], scalar2=None, op0=ALU.mult
                )
                nc.vector.tensor_tensor(out=h_sb[:], in0=w_t[:], in1=n_t[:], op=ALU.mult)
                emit_hT(0)

            def emit_step(t):
                gi_r, gi_z, gi_n = gate_slices(t)
                # gh psum slices, n first (its consumer chain is longest)
                ph_n = php.tile([BL, 512], F32, tag="ph")
                ph_r = php.tile([BL, 512], F32, tag="ph")
                ph_z = php.tile([BL, 512], F32, tag="ph")
                for ns, ph in ((2, ph_n), (0, ph_r), (1, ph_z)):
                    for c in range(4):
                        nc.tensor.matmul(
                            out=ph[:],
                            lhsT=hT[:, BL * c : BL * (c + 1)],
                            rhs=wh[c][:, 512 * ns : 512 * (ns + 1)],
                            start=(c == 0),
                            stop=(c == 3),
                        )
                ghn_bf = kp.tile([BL, V], BF16, tag="ghn_bf")
                if use_biases:
                    nc.vector.tensor_tensor(
                        out=ghn_bf[:], in0=ph_n[:], in1=bhn_mat[:], op=ALU.add
                    )
                else:
                    nc.vector.tensor_copy(out=ghn_bf[:], in_=ph_n[:])
                a_r = kp.tile([BL, V], BF16, tag="a_r")
                nc.vector.tensor_tensor(out=a_r[:], in0=ph_r[:], in1=gi_r, op=ALU.add)
                r_t = kp.tile([BL, V], BF16, tag="r_t")
                nc.scalar.activation(out=r_t[:], in_=a_r[:], func=AF.Sigmoid)
                a_z = kp.tile([BL, V], BF16, tag="a_z")
                nc.vector.tensor_tensor(out=a_z[:], in0=ph_z[:], in1=gi_z, op=ALU.add)
                zp = kp.tile([BL, V], BF16, tag="zp")  # z' = 1-z = sigmoid(-a_z)
                nc.scalar.activation(out=zp[:], in_=a_z[:], func=AF.Sigmoid, scale=-1.0)
                w_t = kp.tile([BL, V], BF16, tag="w_t")  # w = m * z'
                nc.vector.tensor_scalar(
                    out=w_t[:], in0=zp[:], scalar1=tmask[:, t : t + 1], scalar2=None,
                    op0=ALU.mult,
                )
                p_t = kp.tile([BL, V], BF16, tag="p_t")
                nc.vector.tensor_tensor(out=p_t[:], in0=r_t[:], in1=ghn_bf[:], op=ALU.mult)
                a_n = kp.tile([BL, V], BF16, tag="a_n")
                nc.vector.tensor_tensor(out=a_n[:], in0=p_t[:], in1=gi_n, op=ALU.add)
                n_t = kp.tile([BL, V], BF16, tag="n_t")
                nc.scalar.activation(out=n_t[:], in_=a_n[:], func=AF.Tanh)
                u_t = kp.tile([BL, V], BF16, tag="u_t")
                nc.vector.tensor_tensor(out=u_t[:], in0=n_t[:], in1=h_sb[:], op=ALU.subtract)
                t1 = kp.tile([BL, V], BF16, tag="t1")
                nc.vector.tensor_tensor(out=t1[:], in0=w_t[:], in1=u_t[:], op=ALU.mult)
                nc.vector.tensor_tensor(out=h_sb[:], in0=h_sb[:], in1=t1[:], op=ALU.add)
                if t < T - 1:
                    emit_hT(t)

            # interleave gi tiles with early steps so PE stays busy during
            # the gate chains
            emit_gi_tile(0)
            if debug:
                nc.gpsimd.dma_start(out=dbg_ext["dbg_gi0"][:], in_=gi_sb[0][:])
                nc.gpsimd.dma_start(out=dbg_ext["dbg_wi0"][:], in_=wi[0][:])
                nc.gpsimd.dma_start(out=dbg_ext["dbg_xt0"][:], in_=xt[0][:])
            emit_step0()
            if debug:
                nc.gpsimd.dma_start(out=dbg_ext["dbg_h0"][:], in_=h_sb[:])
            emit_gi_tile(1)
            emit_step(1)
            if debug:
                nc.gpsimd.dma_start(out=dbg_ext["dbg_h1"][:], in_=h_sb[:])
            emit_gi_tile(2)
            emit_step(2)
            emit_gi_tile(3)
            for t in range(3, T):
                emit_step(t)

            if debug:
                nc.gpsimd.dma_start(out=dbg_ext["dbg_hF"][:], in_=h_sb[:])
            # ======== logits + out ========
            pl = pgp.tile([BL, V], F32, tag="mm")
            nc.tensor.matmul(
                out=pl[:], lhsT=finT0[:], rhs=wo0[:], start=True, stop=False
            )
            nc.tensor.matmul(
                out=pl[:],
                lhsT=finT1[:],
                rhs=wo1[:],
                start=False,
                stop=not use_biases,
            )
            if use_biases:
                nc.tensor.matmul(
                    out=pl[:], lhsT=ones_row[:, :BL], rhs=bo_bf[:], start=False,
                    stop=True,
                )
            out_f = wp.tile([BL, V], F32, tag="out_f")
            nc.vector.tensor_tensor(out=out_f[:], in0=pl[:], in1=h_sb[:], op=ALU.add)
            nc.sync.dma_start(out=out_ext[:], in_=out_f[:])

            # ======== DDI score partial ========
            s_bf = wp.tile([BL, V], BF16, tag="s_bf")
            nc.scalar.activation(out=s_bf[:], in_=out_f[:], func=AF.Sigmoid)
            sT = wp.tile([128, 4 * BL], BF16, tag="sT")
            pts = ptp.tile([128, 4 * BL], BF16, tag="pt", name="pts")
            for c in range(4):
                nc.tensor.transpose(
                    out=pts[:, BL * c : BL * (c + 1)],
                    in_=s_bf[:, 128 * c : 128 * (c + 1)],
                    identity=ident[:BL, :BL],
                )
            nc.vector.tensor_copy(out=sT[:], in_=pts[:])
            pq = pgp.tile([BL, V], F32, tag="mm")
            for c in range(4):
                nc.tensor.matmul(
                    out=pq[:],
                    lhsT=sT[:, BL * c : BL * (c + 1)],
                    rhs=A_bf[:, V * c : V * (c + 1)],
                    start=(c == 0),
                    stop=(c == 3),
                )
            sq = kp.tile([BL, V], F32, tag="sq")
            nc.vector.tensor_tensor(out=sq[:], in0=pq[:], in1=s_bf[:], op=ALU.mult)
            rcol = kp.tile([BL, 1], F32, tag="rcol")
            nc.vector.tensor_reduce(
                out=rcol[:], in_=sq[:], axis=mybir.AxisListType.X, op=ALU.add
            )
            psc = ptp.tile([1, 1], F32, tag="pt")
            nc.tensor.matmul(
                out=psc[:], lhsT=rcol[:], rhs=ones_col[:], start=True, stop=True
            )
            sc_sb = kp.tile([1, 1], F32, tag="sc_sb")
            nc.vector.tensor_copy(out=sc_sb[:], in_=psc[:])
            nc.sync.dma_start(out=ddi_ext[:], in_=sc_sb[:])

    split_waits(nc, limit=1)
    from concourse.library_overlay import lower_extended_insts

    lower_extended_insts(nc)
    return nc


def split_waits(nc, limit=1):
    """walrus in this toolchain only accepts `limit` sem-waits per
    instruction; move excess waits onto same-engine nops placed before."""
    for f in nc.m.functions:
        for bb in f.blocks:
            insts = list(bb.instructions)
            out = []
            for inst in insts:
                si = inst.sync_info
                waits = list(si.on_wait) if si and si.on_wait else []
                if len(waits) > limit:
                    extra, keep = waits[:-limit], waits[-limit:]
                    for w in extra:
                        nop = nc.engines[inst.engine].nop(nofuse=True).ins
                        for f2 in nc.m.functions:
                            for bb2 in f2.blocks:
                                if nop in list(bb2.instructions):
                                    bb2.instructions.remove(nop)
                        nop.sync_info = mybir.SyncInfo(on_wait=[w], on_update=[])
                        out.append(nop)
                    si.on_wait = keep
                out.append(inst)
            bb.instructions[:] = out


def make_in_maps(inputs):
    """Shard the full inputs into per-core input maps."""
    diagnose = np.ascontiguousarray(np.asarray(inputs["diagnose"]).astype(np.int32))
    procedures = np.ascontiguousarray(np.asarray(inputs["procedures"]).astype(np.int32))
    last_meds = np.ascontiguousarray(np.asarray(inputs["last_meds"]).astype(np.int32))

    def wrap16(a):
        """[BL, S] ints -> dma_gather wrapped idx list [16, BL*S/16] i16,
        flat order i = b + BL*s (column-major), wrapped i -> (i%16, i//16)."""
        flat = a.T.reshape(-1).astype(np.int16)
        return np.ascontiguousarray(flat.reshape(-1, 16).T)
    med_hist = np.ascontiguousarray(np.asarray(inputs["med_hist"], np.float32))
    hist_len = np.ascontiguousarray(
        np.asarray(inputs["hist_len"]).astype(np.int32).reshape(B, 1)
    )
    ddi_adj = np.ascontiguousarray(np.asarray(inputs["ddi_adj"], np.float32))
    diag_table = np.ascontiguousarray(np.asarray(inputs["diag_table"], np.float32))
    proc_table = np.ascontiguousarray(np.asarray(inputs["proc_table"], np.float32))
    med_table = np.ascontiguousarray(np.asarray(inputs["med_table"], np.float32))
    W_out = np.asarray(inputs["W_out"], np.float32)
    W_out = np.ascontiguousarray(
        np.concatenate([W_out, np.zeros((V, E), np.float32)], axis=1)
    )
    b_out = np.ascontiguousarray(np.asarray(inputs["b_out"], np.float32).reshape(1, V))
    W_ih = np.ascontiguousarray(np.asarray(inputs["W_ih"], np.float32))
    W_hh = np.ascontiguousarray(np.asarray(inputs["W_hh"], np.float32))
    b_ih = np.ascontiguousarray(np.asarray(inputs["b_ih"], np.float32).reshape(1, G))
    b_hh = np.ascontiguousarray(np.asarray(inputs["b_hh"], np.float32).reshape(1, G))

    in_maps = []
    for k in range(N_CORES):
        sl = slice(k * BL, (k + 1) * BL)
        in_maps.append(
            dict(
                diag_idx16=wrap16(diagnose[sl]),
                proc_idx16=wrap16(procedures[sl]),
                med_idx16=wrap16(last_meds[sl]),
                last_meds=last_meds[sl],
                med_hist=med_hist[sl],
                hist_len=hist_len[sl],
                ddi_adj=ddi_adj,
                diag_table=diag_table,
                proc_table=proc_table,
                med_table=med_table,
                W_out=W_out,
                b_out=b_out,
                W_ih=W_ih,
                W_hh=W_hh,
                b_ih=b_ih,
                b_hh=b_hh,
            )
        )
    use_biases = bool(
        np.any(b_out) or np.any(b_ih) or np.any(b_hh)
    )
    return in_maps, use_biases


_NC_CACHE = {}


def kernel(**inputs):
    from concourse.bass_utils import run_bass_kernel_spmd

    in_maps, use_biases = make_in_maps(inputs)
    if use_biases not in _NC_CACHE:
        _NC_CACHE[use_biases] = build(use_biases)
    nc = _NC_CACHE[use_biases]
    res = run_bass_kernel_spmd(nc, in_maps, core_ids=list(range(N_CORES)))
    out = np.concatenate([res.results[k]["out"] for k in range(N_CORES)], axis=0)
    total = float(sum(float(res.results[k]["ddi"][0, 0]) for k in range(N_CORES)))
    score = np.float32(KGLOSS_SCALE * total / B)
    return out.astype(np.float32), score


# revision 43
# speedup vs baseline: 1.5525x; 1.5525x over previous
"""Trainium2 Bass kernel for nn_AIModel_34892314312864 (ragged_sequence).

Model (per sample):
  pools  = gather-sum embeddings (diagnose[40], procedures[30], masked last_meds[20])
  logits = concat(pools) @ W_out.T + b_out
  hist   = last hidden of a length-masked GRU over med_hist [T=8, V=512]
  out    = logits + hist
  score  = 0.0005 * mean_b( sigmoid(out)_b @ ddi_adj @ sigmoid(out)_b )

Distribution: pure data parallel, batch 512 sharded 64/core across 8 cores.
Weights replicated. DDI score: per-core partial sums, combined on host.

Key device techniques:
  - Embedding pools: one indirect-DMA gather per table ([64, n*64] f32,
    sample-major) + one strided DVE reduce.
  - All PE compute in bf16. Weights W_ih/W_hh/W_out are transposed to
    v-major during load with zero extra traffic: the f32 HBM tensor is
    viewed as uint16, the odd (high) halves are gathered via the HWDGE
    xbar-transpose DMA; a f32 high-half IS the bf16 truncation.
  - GRU: gi = x @ W_ih.T precomputed for all steps (M=128 matmuls);
    recurrent h @ W_hh.T per step with h kept both sample-major (gate
    math) and v-major (matmul lhsT, via PE transposes).
"""

import sys

sys.path.insert(0, "/opt/trn_rl_repo")

import numpy as np

from concourse import bass, library_config, mybir
from concourse.tile import TileContext, add_dep_helper

F32 = mybir.dt.float32
BF16 = mybir.dt.bfloat16
I32 = mybir.dt.int32
I16 = mybir.dt.int16
U16 = mybir.dt.uint16
AF = mybir.ActivationFunctionType
ALU = mybir.AluOpType

# Problem constants (hardcoded; kernel.py must be self-contained)
B, D, P, M, T, V, E = 512, 40, 30, 20, 8, 512, 64
DV, PV = 2000, 1500
MED_PAD = V + 2  # 514
G = 3 * V  # 1536
N_CORES = 8
BL = B // N_CORES  # 64 samples per core
KGLOSS_SCALE = 0.001 * 0.5

DIAG_ROWS, PROC_ROWS, MED_ROWS = DV + 3, PV + 3, V + 3


def _u16_hi_cols(ap_f32, col0, col1):
    """View f32 DRAM AP as uint16 and select the high halves of f32
    columns [col0, col1) -> shape [rows, col1-col0] u16 (strided)."""
    u = ap_f32.bitcast(U16)
    return u[:, 2 * col0 + 1 : 2 * col1 : 2]


def build(use_biases: bool, debug: bool = False):
    nc = bass.Bass()
    dbg_ext = {}
    if debug:
        for nm, shape in [
            ("dbg_fin", [BL, 3 * E]),
            ("dbg_gi0", [128, G]),
            ("dbg_h0", [BL, V]),
            ("dbg_h1", [BL, V]),
            ("dbg_hF", [BL, V]),
            ("dbg_wi0", [128, G]),
            ("dbg_xt0", [128, 512]),
        ]:
            dbg_ext[nm] = nc.declare_dram_parameter(nm, shape, F32, isOutput=True)

    # ---- parameters (per-core shards / replicated weights) ----
    diag_idx16 = nc.declare_dram_parameter("diag_idx16", [16, BL * D // 16], I16, isOutput=False)
    proc_idx16 = nc.declare_dram_parameter("proc_idx16", [16, BL * P // 16], I16, isOutput=False)
    med_idx16 = nc.declare_dram_parameter("med_idx16", [16, BL * M // 16], I16, isOutput=False)
    last_meds = nc.declare_dram_parameter("last_meds", [BL, M], I32, isOutput=False)
    # med_hist is passed t-major ([T, BL, V]) so the bf16 cast-DMA is contiguous
    med_hist = nc.declare_dram_parameter("med_hist", [T, BL, V], F32, isOutput=False)
    hist_len = nc.declare_dram_parameter("hist_len", [BL, 1], I32, isOutput=False)
    ddi_adj = nc.declare_dram_parameter("ddi_adj", [V, V], F32, isOutput=False)
    diag_table = nc.declare_dram_parameter("diag_table", [DIAG_ROWS, E], F32, isOutput=False)
    proc_table = nc.declare_dram_parameter("proc_table", [PROC_ROWS, E], F32, isOutput=False)
    med_table = nc.declare_dram_parameter("med_table", [MED_ROWS, E], F32, isOutput=False)
    # W_out is zero-padded on host from [V, 192] to [V, 256] so its transpose
    # splits into two full 128-row chunks
    W_out = nc.declare_dram_parameter("W_out", [V, 4 * E], F32, isOutput=False)
    b_out = nc.declare_dram_parameter("b_out", [1, V], F32, isOutput=False)
    W_ih = nc.declare_dram_parameter("W_ih", [G, V], F32, isOutput=False)
    W_hh = nc.declare_dram_parameter("W_hh", [G, V], F32, isOutput=False)
    b_ih = nc.declare_dram_parameter("b_ih", [1, G], F32, isOutput=False)
    b_hh = nc.declare_dram_parameter("b_hh", [1, G], F32, isOutput=False)
    out_ext = nc.declare_dram_parameter("out", [BL, V], F32, isOutput=True)
    ddi_ext = nc.declare_dram_parameter("ddi", [1, 1], F32, isOutput=True)

    # ---- inline constants ----
    import ml_dtypes

    ident_np = np.zeros((128, 128), np.float32)
    np.fill_diagonal(ident_np, 1.0)

    ident_c = nc.inline_tensor(ident_np.astype(ml_dtypes.bfloat16), name="ident_bf")
    iota_c = nc.inline_tensor(
        np.tile(np.arange(T, dtype=np.float32), (BL, 1)), name="iota8"
    )
    ones_col_c = nc.inline_tensor(np.ones((BL, 1), np.float32), name="ones_col")
    # iota2c[p, c] = 2c + (p >= 64): the slot number at (partition, chunk)
    iota2c_np = (2 * np.arange(M // 2, dtype=np.float32))[None, :] + (
        np.arange(128) >= BL
    ).astype(np.float32)[:, None]
    iota2c_c = nc.inline_tensor(iota2c_np, name="iota2c")
    # pairsel[p, b] = 1 if p % 64 == b (sums partition b and 64+b)
    pairsel_np = np.zeros((128, BL), np.float32)
    pairsel_np[np.arange(128), np.arange(128) % BL] = 1.0
    pairsel_c = nc.inline_tensor(pairsel_np.astype(ml_dtypes.bfloat16), name="pairsel")
    if use_biases:
        ones_row_c = nc.inline_tensor(
            np.ones((1, 128), ml_dtypes.bfloat16), name="ones_row"
        )
        # mask that keeps r,z parts of b_hh and zeroes the n part
        rz_np = np.zeros((1, G), np.float32)
        rz_np[:, : 2 * V] = 1.0
        rz_mask_c = nc.inline_tensor(rz_np, name="rz_mask")

    with TileContext(nc) as tc:
        with (
            tc.tile_pool(name="wts", bufs=1) as wp,
            tc.tile_pool(name="work", bufs=2) as kp,
            tc.tile_pool(name="gath", bufs=1) as gp,
            tc.tile_pool(name="psum", bufs=2, space="PSUM") as pgp,
            tc.tile_pool(name="psum_h", bufs=4, space="PSUM") as php,
            tc.tile_pool(name="psum_t", bufs=2, space="PSUM") as ptp,
        ):
            # PSUM budget (8 banks): pgp tag "mm" [128,512] x3 + php tag
            # "ph" [64,512] x3 + ptp tag "pt" [128,128] x2 = 8.
            nc.gpsimd.load_library(library_config.mlp)
            # ======== constants to SBUF ========
            ident = wp.tile([128, 128], BF16, tag="ident")
            nc.scalar.dma_start(out=ident[:], in_=ident_c[:])
            iota = wp.tile([BL, T], F32, tag="iota")
            nc.scalar.dma_start(out=iota[:], in_=iota_c[:])
            ones_col = wp.tile([BL, 1], F32, tag="ones_col")
            nc.scalar.dma_start(out=ones_col[:], in_=ones_col_c[:])
            iota2c = wp.tile([128, M // 2], F32, tag="iota2c")
            nc.scalar.dma_start(out=iota2c[:], in_=iota2c_c[:])
            pairsel = wp.tile([128, BL], BF16, tag="pairsel")
            nc.scalar.dma_start(out=pairsel[:], in_=pairsel_c[:])

            # ======== weight loads ========
            # x and W_ih are prologue-critical: f32 HWDGE load + DVE cast +
            # PE transpose (no DRAM round-trip). W_hh/W_out bounce through a
            # bf16 DRAM scratch and xbar-transpose in (hidden under gi).
            whh_bf = nc.dram_tensor("whh_bf", [G, V], BF16)
            wout_bf = nc.dram_tensor("wout_bf", [V, 4 * E], BF16)
            nc.gpsimd.dma_start(out=whh_bf[:], in_=W_hh[:])
            nc.gpsimd.dma_start(out=wout_bf[:], in_=W_out[:])

            wi = [wp.tile([128, G], BF16, tag=f"wi{c}", name=f"wi{c}") for c in range(4)]
            wh = [wp.tile([128, G], BF16, tag=f"wh{c}", name=f"wh{c}") for c in range(4)]
            xt = [wp.tile([128, T * BL], BF16, tag=f"xt{c}", name=f"xt{c}") for c in range(4)]
            wo0 = wp.tile([128, V], BF16, tag="wo0")
            wo1 = wp.tile([128, V], BF16, tag="wo1")

            def emit_load_T(src_dram, n_rows, dst_chunks, n_fchunks, tmp_tag):
                for k in range(n_rows):
                    bf = kp.tile([128, n_fchunks * 128], F32, tag=f"{tmp_tag}f",
                                 name=f"{tmp_tag}f{k}")
                    nc.sync.dma_start(
                        out=bf[:], in_=src_dram[128 * k : 128 * (k + 1), :]
                    )
                    bb_ = kp.tile([128, n_fchunks * 128], BF16, tag=f"{tmp_tag}b",
                                  name=f"{tmp_tag}b{k}")
                    nc.vector.tensor_copy(out=bb_[:], in_=bf[:])
                    ptw = ptp.tile([128, n_fchunks * 128], BF16, tag="pt",
                                   name=f"pt{tmp_tag}{k}")
                    for c in range(n_fchunks):
                        nc.tensor.transpose(
                            out=ptw[:, 128 * c : 128 * (c + 1)],
                            in_=bb_[:, 128 * c : 128 * (c + 1)],
                            identity=ident[:],
                        )
                    for c in range(n_fchunks):
                        nc.vector.tensor_copy(
                            out=dst_chunks[c][:, 128 * k : 128 * (k + 1)],
                            in_=ptw[:, 128 * c : 128 * (c + 1)],
                        )

            emit_load_T(med_hist[:].rearrange("t b v -> (t b) v"), 4, xt, 4, "xld")
            emit_load_T(W_ih[:], 12, wi, 4, "wld")

            xbar_insts = []
            for c in range(4):
                xbar_insts.append(nc.sync.dma_start(
                    out=wh[c][:], in_=whh_bf[:, 128 * c : 128 * (c + 1)], transpose=True
                ))
            xbar_insts.append(
                nc.sync.dma_start(out=wo0[:], in_=wout_bf[:, 0:128], transpose=True)
            )
            xbar_insts.append(
                nc.sync.dma_start(out=wo1[:], in_=wout_bf[:, 128:256], transpose=True)
            )
            hwdge_late = []  # plain HWDGE copies to push after the last xbar

            # ======== hist-len mask ========
            hl_i = gp.tile([BL, 1], I32, tag="hl_i")
            nc.scalar.dma_start(out=hl_i[:], in_=hist_len[:])
            hl_f = gp.tile([BL, 1], F32, tag="hl_f")
            nc.vector.tensor_copy(out=hl_f[:], in_=hl_i[:])
            tmask = wp.tile([BL, T], F32, tag="tmask")
            nc.vector.tensor_scalar(
                out=tmask[:], in0=iota[:], scalar1=hl_f[:], scalar2=None, op0=ALU.is_lt
            )

            # pool-section compute must not head-of-line block the PE/DVE
            # streams mid-recurrence: push it after the final h update
            _pool_sec_end = len(nc.cur_bb.bb.instructions)
            if last_h[0] is not None:
                _sec = list(nc.cur_bb.bb.instructions)[_pool_sec_start:_pool_sec_end]
                for _inst in _sec:
                    eng = str(_inst.engine)
                    if eng.endswith("PE") or eng.endswith("DVE"):
                        add_dep_helper(
                            _inst, last_h[0].ins, reason="defer pool compute past GRU"
                        )
            for d in hwdge_late:
                add_dep_helper(d.ins, xbar_insts[-1].ins, reason="defer HWDGE copy past xbars")

            # ======== ddi_adj -> bf16 (one cast-DMA) ========
            A_bf = wp.tile([128, 4 * V], BF16, tag="abf")
            nc.gpsimd.dma_start(
                out=A_bf[:], in_=ddi_adj[:].rearrange("(k p) v -> p k v", p=128)
            )

            # ======== biases ========
            if use_biases:
                ones_row = wp.tile([1, 128], BF16, tag="ones_row")
                nc.scalar.dma_start(out=ones_row[:], in_=ones_row_c[:])
                bo_bf = wp.tile([1, V], BF16, tag="bo_bf")
                nc.gpsimd.dma_start(out=bo_bf[:], in_=b_out[:])
                # gi bias = b_ih + rz_mask * b_hh ; n-part of b_hh added per step
                bi_f = wp.tile([1, G], F32, tag="bi_f")
                nc.scalar.dma_start(out=bi_f[:], in_=b_ih[:])
                bh_f = wp.tile([1, G], F32, tag="bh_f")
                nc.scalar.dma_start(out=bh_f[:], in_=b_hh[:])
                rz_m = wp.tile([1, G], F32, tag="rz_m")
                nc.scalar.dma_start(out=rz_m[:], in_=rz_mask_c[:])
                gi_bias = wp.tile([1, G], F32, tag="gi_bias")
                nc.vector.tensor_tensor(
                    out=gi_bias[:], in0=bh_f[:], in1=rz_m[:], op=ALU.mult
                )
                nc.vector.tensor_tensor(
                    out=gi_bias[:], in0=gi_bias[:], in1=bi_f[:], op=ALU.add
                )
                gi_bias_bf = wp.tile([1, G], BF16, tag="gi_bias_bf")
                nc.vector.tensor_copy(out=gi_bias_bf[:], in_=gi_bias[:])
                bh_n_bf = wp.tile([1, V], BF16, tag="bh_n_bf")
                nc.vector.tensor_copy(out=bh_n_bf[:], in_=bh_f[:, 2 * V : 3 * V])
                # materialize b_hh_n broadcast to all samples (K=1 matmul)
                pbh = pgp.tile([BL, V], F32, tag="mm")
                nc.tensor.matmul(
                    out=pbh[:], lhsT=ones_row[:, :BL], rhs=bh_n_bf[:],
                    start=True, stop=True,
                )
                bhn_mat = wp.tile([BL, V], BF16, tag="bhn_mat")
                nc.vector.tensor_copy(out=bhn_mat[:], in_=pbh[:])

            # ======== gi = x @ W_ih.T (+ folded biases), all 8 steps ========
            gi_sb = [wp.tile([128, G], BF16, tag=f"gi{j}", name=f"gi{j}") for j in range(4)]
            # odd steps live at partition base 64 inside gi_sb; compute
            # engines need base-0 operands, so DMA the odd halves down


            def emit_gi_tile(j):
                for ns in range(3):
                    pg = pgp.tile([128, 512], F32, tag="mm")
                    for c in range(4):
                        nc.tensor.matmul(
                            out=pg[:],
                            lhsT=xt[c][:, 128 * j : 128 * (j + 1)],
                            rhs=wi[c][:, 512 * ns : 512 * (ns + 1)],
                            start=(c == 0),
                            stop=(c == 3) if not use_biases else False,
                        )
                    if use_biases:
                        nc.tensor.matmul(
                            out=pg[:],
                            lhsT=ones_row[:, :128],
                            rhs=gi_bias_bf[:, 512 * ns : 512 * (ns + 1)],
                            start=False,
                            stop=True,
                        )
                    nc.scalar.activation(
                        out=gi_sb[j][:, 512 * ns : 512 * (ns + 1)], in_=pg[:],
                        func=AF.Copy,
                    )

            # ======== GRU recurrence ========
            gi_odd = [
                wp.tile([BL, V], BF16, tag=f"giodd{j}", name=f"giodd{j}")
                for j in range(4)
            ]
            h_sb = wp.tile([BL, V], BF16, tag="h_sb")
            hT = wp.tile([128, 4 * BL], BF16, tag="hT")

            last_h = [None]

            def gate_slices(t):
                """gi slices for step t; odd steps live at partition base 64
                and are only consumed by base-64 identity matmuls."""
                j, half = t // 2, (t % 2) * BL
                sl = gi_sb[j][half : half + BL, :]
                return sl[:, 0:V], sl[:, V : 2 * V], sl[:, 2 * V : 3 * V]

            def emit_hT(t):
                ptt = ptp.tile([128, 4 * BL], BF16, tag="pt", name=f"ptt{t}")
                for c in range(4):
                    nc.tensor.transpose(
                        out=ptt[:, BL * c : BL * (c + 1)],
                        in_=h_sb[:, 128 * c : 128 * (c + 1)],
                        identity=ident[:BL, :BL],
                    )
                nc.vector.tensor_copy(out=hT[:], in_=ptt[:])

            def _gi_odd_dma(j):
                hwdge_late.append(nc.sync.dma_start(
                    out=gi_odd[j][:], in_=gi_sb[j][BL:128, 2 * V : 3 * V]
                ))

            def emit_step0():
                gi_r, gi_z, gi_n = gate_slices(0)
                zp = kp.tile([BL, V], BF16, tag="zp")
                nc.scalar.activation(out=zp[:], in_=gi_z, func=AF.Sigmoid, scale=-1.0)
                n_t = kp.tile([BL, V], BF16, tag="n_t")
                if use_biases:
                    # n = tanh(gi_n + sigmoid(gi_r) * b_hh_n)   (h0 = 0)
                    r_t = kp.tile([BL, V], BF16, tag="r_t")
                    nc.scalar.activation(out=r_t[:], in_=gi_r, func=AF.Sigmoid)
                    p_t = kp.tile([BL, V], BF16, tag="p_t")
                    nc.vector.tensor_tensor(
                        out=p_t[:], in0=r_t[:], in1=bhn_mat[:], op=ALU.mult
                    )
                    a_n = kp.tile([BL, V], BF16, tag="a_n")
                    nc.vector.tensor_tensor(out=a_n[:], in0=p_t[:], in1=gi_n, op=ALU.add)
                    nc.scalar.activation(out=n_t[:], in_=a_n[:], func=AF.Tanh)
                else:
                    nc.scalar.activation(out=n_t[:], in_=gi_n, func=AF.Tanh)
                # h = m * z' * n
                w_t = kp.tile([BL, V], BF16, tag="w_t")
                nc.vector.tensor_scalar(
                    out=w_t[:], in0=zp[:], scalar1=tmask[:, 0:1], scalar2=None, op0=ALU.mult
                )
                nc.vector.tensor_tensor(out=h_sb[:], in0=w_t[:], in1=n_t[:], op=ALU.mult)
                emit_hT(0)

            def emit_step(t):
                gi_r, gi_z, gi_n = gate_slices(t)
                # gh psum slices, n first (its consumer chain is longest)
                ph_n = php.tile([BL, 512], F32, tag="ph")
                ph_r = php.tile([BL, 512], F32, tag="ph")
                ph_z = php.tile([BL, 512], F32, tag="ph")
                # gh psum accumulations; the r/z slices also fold the gi add
                # in via an extra K=64 identity matmul so ACT can consume the
                # PSUM directly (saves a DVE add + a hop per gate)
                for ns, ph, gi_sl in ((2, ph_n, None), (0, ph_r, gi_r), (1, ph_z, gi_z)):
                    for c in range(4):
                        nc.tensor.matmul(
                            out=ph[:],
                            lhsT=hT[:, BL * c : BL * (c + 1)],
                            rhs=wh[c][:, 512 * ns : 512 * (ns + 1)],
                            start=(c == 0),
                            stop=(c == 3) and gi_sl is None,
                        )
                    if gi_sl is not None:
                        idnt = (
                            ident[:BL, :BL]
                            if t % 2 == 0
                            else ident[BL:128, BL:128]
                        )
                        nc.tensor.matmul(
                            out=ph[:], lhsT=idnt, rhs=gi_sl,
                            start=False, stop=True,
                        )
                r_t = kp.tile([BL, V], BF16, tag="r_t")
                nc.scalar.activation(out=r_t[:], in_=ph_r[:], func=AF.Sigmoid)
                zp = kp.tile([BL, V], BF16, tag="zp")  # z' = 1-z = sigmoid(-a_z)
                nc.scalar.activation(out=zp[:], in_=ph_z[:], func=AF.Sigmoid, scale=-1.0)
                w_t = kp.tile([BL, V], BF16, tag="w_t")  # w = m * z'
                nc.vector.tensor_scalar(
                    out=w_t[:], in0=zp[:], scalar1=tmask[:, t : t + 1], scalar2=None,
                    op0=ALU.mult,
                )
                p_t = kp.tile([BL, V], BF16, tag="p_t")
                if use_biases:
                    ghn_bf = kp.tile([BL, V], BF16, tag="ghn_bf")
                    nc.vector.tensor_tensor(
                        out=ghn_bf[:], in0=ph_n[:], in1=bhn_mat[:], op=ALU.add
                    )
                    nc.vector.tensor_tensor(
                        out=p_t[:], in0=r_t[:], in1=ghn_bf[:], op=ALU.mult
                    )
                else:
                    nc.vector.tensor_tensor(
                        out=p_t[:], in0=ph_n[:], in1=r_t[:], op=ALU.mult
                    )
                gi_n_b0 = gi_n if t % 2 == 0 else gi_odd[t // 2][:]
                a_n = kp.tile([BL, V], BF16, tag="a_n")
                nc.vector.tensor_tensor(out=a_n[:], in0=p_t[:], in1=gi_n_b0, op=ALU.add)
                n_t = kp.tile([BL, V], BF16, tag="n_t")
                nc.scalar.activation(out=n_t[:], in_=a_n[:], func=AF.Tanh)
                u_t = kp.tile([BL, V], BF16, tag="u_t")
                nc.vector.tensor_tensor(out=u_t[:], in0=n_t[:], in1=h_sb[:], op=ALU.subtract)
                t1 = kp.tile([BL, V], BF16, tag="t1")
                nc.vector.tensor_tensor(out=t1[:], in0=w_t[:], in1=u_t[:], op=ALU.mult)
                last_h[0] = nc.vector.tensor_tensor(
                    out=h_sb[:], in0=h_sb[:], in1=t1[:], op=ALU.add
                )
                if t < T - 1:
                    emit_hT(t)

            # interleave gi tiles with early steps so PE stays busy during
            # the gate chains
            emit_gi_tile(0)
            _gi_odd_dma(0)
            if debug:
                nc.gpsimd.dma_start(out=dbg_ext["dbg_gi0"][:], in_=gi_sb[0][:])
                nc.gpsimd.dma_start(out=dbg_ext["dbg_wi0"][:], in_=wi[0][:])
                nc.gpsimd.dma_start(out=dbg_ext["dbg_xt0"][:], in_=xt[0][:])
            emit_step0()
            if debug:
                nc.gpsimd.dma_start(out=dbg_ext["dbg_h0"][:], in_=h_sb[:])
            emit_gi_tile(1)
            _gi_odd_dma(1)
            emit_step(1)
            if debug:
                nc.gpsimd.dma_start(out=dbg_ext["dbg_h1"][:], in_=h_sb[:])
            emit_gi_tile(2)
            _gi_odd_dma(2)
            emit_step(2)
            emit_gi_tile(3)
            _gi_odd_dma(3)
            for t in range(3, T):
                emit_step(t)

            if debug:
                nc.gpsimd.dma_start(out=dbg_ext["dbg_hF"][:], in_=h_sb[:])
            # ======== logits + out ========
            pl = pgp.tile([BL, V], F32, tag="mm")
            nc.tensor.matmul(
                out=pl[:], lhsT=finT0[:], rhs=wo0[:], start=True, stop=False
            )
            nc.tensor.matmul(
                out=pl[:],
                lhsT=finT1[:],
                rhs=wo1[:],
                start=False,
                stop=not use_biases,
            )
            if use_biases:
                nc.tensor.matmul(
                    out=pl[:], lhsT=ones_row[:, :BL], rhs=bo_bf[:], start=False,
                    stop=True,
                )
            out_f = wp.tile([BL, V], F32, tag="out_f")
            nc.vector.tensor_tensor(out=out_f[:], in0=pl[:], in1=h_sb[:], op=ALU.add)
            nc.sync.dma_start(out=out_ext[:], in_=out_f[:])

            # ======== DDI score partial ========
            s_bf = wp.tile([BL, V], BF16, tag="s_bf")
            nc.scalar.activation(out=s_bf[:], in_=out_f[:], func=AF.Sigmoid)
            sT = wp.tile([128, 4 * BL], BF16, tag="sT")
            pts = ptp.tile([128, 4 * BL], BF16, tag="pt", name="pts")
            for c in range(4):
                nc.tensor.transpose(
                    out=pts[:, BL * c : BL * (c + 1)],
                    in_=s_bf[:, 128 * c : 128 * (c + 1)],
                    identity=ident[:BL, :BL],
                )
            nc.vector.tensor_copy(out=sT[:], in_=pts[:])
            pq = pgp.tile([BL, V], F32, tag="mm")
            for c in range(4):
                nc.tensor.matmul(
                    out=pq[:],
                    lhsT=sT[:, BL * c : BL * (c + 1)],
                    rhs=A_bf[:, V * c : V * (c + 1)],
                    start=(c == 0),
                    stop=(c == 3),
                )
            sq = kp.tile([BL, V], F32, tag="sq")
            nc.vector.tensor_tensor(out=sq[:], in0=pq[:], in1=s_bf[:], op=ALU.mult)
            rcol = kp.tile([BL, 1], F32, tag="rcol")
            nc.vector.tensor_reduce(
                out=rcol[:], in_=sq[:], axis=mybir.AxisListType.X, op=ALU.add
            )
            psc = ptp.tile([1, 1], F32, tag="pt")
            nc.tensor.matmul(
                out=psc[:], lhsT=rcol[:], rhs=ones_col[:], start=True, stop=True
            )
            sc_sb = kp.tile([1, 1], F32, tag="sc_sb")
            nc.vector.tensor_copy(out=sc_sb[:], in_=psc[:])
            nc.sync.dma_start(out=ddi_ext[:], in_=sc_sb[:])

    split_waits(nc, limit=1)
    from concourse.library_overlay import lower_extended_insts

    lower_extended_insts(nc)
    return nc


def split_waits(nc, limit=1):
    """walrus in this toolchain only accepts `limit` sem-waits per
    instruction; move excess waits onto same-engine nops placed before."""
    for f in nc.m.functions:
        for bb in f.blocks:
            insts = list(bb.instructions)
            out = []
            for inst in insts:
                si = inst.sync_info
                waits = list(si.on_wait) if si and si.on_wait else []
                if len(waits) > limit:
                    extra, keep = waits[:-limit], waits[-limit:]
                    for w in extra:
                        nop = nc.engines[inst.engine].nop(nofuse=True).ins
                        for f2 in nc.m.functions:
                            for bb2 in f2.blocks:
                                if nop in list(bb2.instructions):
                                    bb2.instructions.remove(nop)
                        nop.sync_info = mybir.SyncInfo(on_wait=[w], on_update=[])
                        out.append(nop)
                    si.on_wait = keep
                out.append(inst)
            bb.instructions[:] = out


def make_in_maps(inputs):
    """Shard the full inputs into per-core input maps."""
    diagnose = np.ascontiguousarray(np.asarray(inputs["diagnose"]).astype(np.int32))
    procedures = np.ascontiguousarray(np.asarray(inputs["procedures"]).astype(np.int32))
    last_meds = np.ascontiguousarray(np.asarray(inputs["last_meds"]).astype(np.int32))

    def wrap16(a):
        """[BL, S] ints -> dma_gather wrapped idx list [16, BL*S/16] i16,
        flat order i = b + BL*s (column-major), wrapped i -> (i%16, i//16)."""
        flat = a.T.reshape(-1).astype(np.int16)
        return np.ascontiguousarray(flat.reshape(-1, 16).T)
    med_hist = np.ascontiguousarray(np.asarray(inputs["med_hist"], np.float32))
    hist_len = np.ascontiguousarray(
        np.asarray(inputs["hist_len"]).astype(np.int32).reshape(B, 1)
    )
    ddi_adj = np.ascontiguousarray(np.asarray(inputs["ddi_adj"], np.float32))
    diag_table = np.ascontiguousarray(np.asarray(inputs["diag_table"], np.float32))
    proc_table = np.ascontiguousarray(np.asarray(inputs["proc_table"], np.float32))
    med_table = np.ascontiguousarray(np.asarray(inputs["med_table"], np.float32))
    W_out = np.asarray(inputs["W_out"], np.float32)
    W_out = np.ascontiguousarray(
        np.concatenate([W_out, np.zeros((V, E), np.float32)], axis=1)
    )
    b_out = np.ascontiguousarray(np.asarray(inputs["b_out"], np.float32).reshape(1, V))
    W_ih = np.ascontiguousarray(np.asarray(inputs["W_ih"], np.float32))
    W_hh = np.ascontiguousarray(np.asarray(inputs["W_hh"], np.float32))
    b_ih = np.ascontiguousarray(np.asarray(inputs["b_ih"], np.float32).reshape(1, G))
    b_hh = np.ascontiguousarray(np.asarray(inputs["b_hh"], np.float32).reshape(1, G))

    in_maps = []
    for k in range(N_CORES):
        sl = slice(k * BL, (k + 1) * BL)
        in_maps.append(
            dict(
                diag_idx16=wrap16(diagnose[sl]),
                proc_idx16=wrap16(procedures[sl]),
                med_idx16=wrap16(last_meds[sl]),
                last_meds=last_meds[sl],
                med_hist=np.ascontiguousarray(med_hist[sl].transpose(1, 0, 2)),
                hist_len=hist_len[sl],
                ddi_adj=ddi_adj,
                diag_table=diag_table,
                proc_table=proc_table,
                med_table=med_table,
                W_out=W_out,
                b_out=b_out,
                W_ih=W_ih,
                W_hh=W_hh,
                b_ih=b_ih,
                b_hh=b_hh,
            )
        )
    use_biases = bool(
        np.any(b_out) or np.any(b_ih) or np.any(b_hh)
    )
    return in_maps, use_biases


_NC_CACHE = {}


def kernel(**inputs):
    from concourse.bass_utils import run_bass_kernel_spmd

    in_maps, use_biases = make_in_maps(inputs)
    if use_biases not in _NC_CACHE:
        _NC_CACHE[use_biases] = build(use_biases)
    nc = _NC_CACHE[use_biases]
    res = run_bass_kernel_spmd(nc, in_maps, core_ids=list(range(N_CORES)))
    out = np.concatenate([res.results[k]["out"] for k in range(N_CORES)], axis=0)
    total = float(sum(float(res.results[k]["ddi"][0, 0]) for k in range(N_CORES)))
    score = np.float32(KGLOSS_SCALE * total / B)
    return out.astype(np.float32), score


# revision 45
# speedup vs baseline: 1.6774x; 1.0804x over previous
"""Trainium2 Bass kernel for nn_AIModel_34892314312864 (ragged_sequence).

Model (per sample):
  pools  = gather-sum embeddings (diagnose[40], procedures[30], masked last_meds[20])
  logits = concat(pools) @ W_out.T + b_out
  hist   = last hidden of a length-masked GRU over med_hist [T=8, V=512]
  out    = logits + hist
  score  = 0.0005 * mean_b( sigmoid(out)_b @ ddi_adj @ sigmoid(out)_b )

Distribution: pure data parallel, batch 512 sharded 64/core across 8 cores.
Weights replicated. DDI score: per-core partial sums, combined on host.

Key device techniques:
  - Embedding pools: one dma_gather per table with a column-major int16
    index list (sample b's even/odd slots land in partitions b / b+64,
    slots on the free dim), masked strided DVE reduce, one pair-sum
    matmul. The gathers run on the Q7 queue fully hidden under the GRU.
  - All PE compute in bf16. x and W_ih (prologue-critical) load as f32 +
    DVE cast + PE transposes; W_hh/W_out bounce through a bf16 DRAM
    scratch and xbar-transpose in, hidden under the gi matmuls.
  - GRU: gi = x @ W_ih.T precomputed for all 8 steps (M=128 matmuls)
    and interleaved into the recurrence to keep the PE warm; per step
    the gi r/z adds are folded into the gh PSUM accumulation as K=64
    identity matmuls (base-64 identities fix odd-step partition bases),
    so sigmoids read PSUM directly; gates run on ACT, elementwise on
    DVE; h is re-transposed each step via packed PE transposes.
  - Pool/logits/DDI compute is dependency-deferred behind the final h
    update so it cannot head-of-line block the PE/DVE streams.
"""

import sys

sys.path.insert(0, "/opt/trn_rl_repo")

import numpy as np

from concourse import bass, library_config, mybir
from concourse.tile import TileContext, add_dep_helper

F32 = mybir.dt.float32
BF16 = mybir.dt.bfloat16
I32 = mybir.dt.int32
I16 = mybir.dt.int16
U16 = mybir.dt.uint16
AF = mybir.ActivationFunctionType
ALU = mybir.AluOpType

# Problem constants (hardcoded; kernel.py must be self-contained)
B, D, P, M, T, V, E = 512, 40, 30, 20, 8, 512, 64
DV, PV = 2000, 1500
MED_PAD = V + 2  # 514
G = 3 * V  # 1536
N_CORES = 8
BL = B // N_CORES  # 64 samples per core
KGLOSS_SCALE = 0.001 * 0.5

DIAG_ROWS, PROC_ROWS, MED_ROWS = DV + 3, PV + 3, V + 3


def build(use_biases: bool, debug: bool = False):
    nc = bass.Bass()
    dbg_ext = {}
    if debug:
        for nm, shape in [
            ("dbg_fin", [BL, 3 * E]),
            ("dbg_gi0", [128, G]),
            ("dbg_h0", [BL, V]),
            ("dbg_h1", [BL, V]),
            ("dbg_hF", [BL, V]),
            ("dbg_wi0", [128, G]),
            ("dbg_xt0", [128, 512]),
        ]:
            dbg_ext[nm] = nc.declare_dram_parameter(nm, shape, F32, isOutput=True)

    # ---- parameters (per-core shards / replicated weights) ----
    diag_idx16 = nc.declare_dram_parameter("diag_idx16", [16, BL * D // 16], I16, isOutput=False)
    proc_idx16 = nc.declare_dram_parameter("proc_idx16", [16, BL * P // 16], I16, isOutput=False)
    med_idx16 = nc.declare_dram_parameter("med_idx16", [16, BL * M // 16], I16, isOutput=False)
    last_meds = nc.declare_dram_parameter("last_meds", [BL, M], I32, isOutput=False)
    # med_hist is passed t-major ([T, BL, V]) so the bf16 cast-DMA is contiguous
    med_hist = nc.declare_dram_parameter("med_hist", [T, BL, V], F32, isOutput=False)
    hist_len = nc.declare_dram_parameter("hist_len", [BL, 1], I32, isOutput=False)
    ddi_adj = nc.declare_dram_parameter("ddi_adj", [V, V], F32, isOutput=False)
    diag_table = nc.declare_dram_parameter("diag_table", [DIAG_ROWS, E], F32, isOutput=False)
    proc_table = nc.declare_dram_parameter("proc_table", [PROC_ROWS, E], F32, isOutput=False)
    med_table = nc.declare_dram_parameter("med_table", [MED_ROWS, E], F32, isOutput=False)
    # W_out is zero-padded on host from [V, 192] to [V, 256] so its transpose
    # splits into two full 128-row chunks
    W_out = nc.declare_dram_parameter("W_out", [V, 4 * E], F32, isOutput=False)
    b_out = nc.declare_dram_parameter("b_out", [1, V], F32, isOutput=False)
    W_ih = nc.declare_dram_parameter("W_ih", [G, V], F32, isOutput=False)
    W_hh = nc.declare_dram_parameter("W_hh", [G, V], F32, isOutput=False)
    b_ih = nc.declare_dram_parameter("b_ih", [1, G], F32, isOutput=False)
    b_hh = nc.declare_dram_parameter("b_hh", [1, G], F32, isOutput=False)
    out_ext = nc.declare_dram_parameter("out", [BL, V], F32, isOutput=True)
    ddi_ext = nc.declare_dram_parameter("ddi", [1, 1], F32, isOutput=True)

    # ---- inline constants ----
    import ml_dtypes

    ident_np = np.zeros((128, 128), np.float32)
    np.fill_diagonal(ident_np, 1.0)

    ident_c = nc.inline_tensor(ident_np.astype(ml_dtypes.bfloat16), name="ident_bf")
    iota_c = nc.inline_tensor(
        np.tile(np.arange(T, dtype=np.float32), (BL, 1)), name="iota8"
    )
    ones_col_c = nc.inline_tensor(np.ones((BL, 1), np.float32), name="ones_col")
    # iota2c[p, c] = 2c + (p >= 64): the slot number at (partition, chunk)
    iota2c_np = (2 * np.arange(M // 2, dtype=np.float32))[None, :] + (
        np.arange(128) >= BL
    ).astype(np.float32)[:, None]
    iota2c_c = nc.inline_tensor(iota2c_np, name="iota2c")
    # pairsel[p, b] = 1 if p % 64 == b (sums partition b and 64+b)
    pairsel_np = np.zeros((128, BL), np.float32)
    pairsel_np[np.arange(128), np.arange(128) % BL] = 1.0
    pairsel_c = nc.inline_tensor(pairsel_np.astype(ml_dtypes.bfloat16), name="pairsel")
    if use_biases:
        ones_row_c = nc.inline_tensor(
            np.ones((1, 128), ml_dtypes.bfloat16), name="ones_row"
        )
        # mask that keeps r,z parts of b_hh and zeroes the n part
        rz_np = np.zeros((1, G), np.float32)
        rz_np[:, : 2 * V] = 1.0
        rz_mask_c = nc.inline_tensor(rz_np, name="rz_mask")

    with TileContext(nc) as tc:
        with (
            tc.tile_pool(name="wts", bufs=1) as wp,
            tc.tile_pool(name="work", bufs=2) as kp,
            tc.tile_pool(name="gath", bufs=1) as gp,
            tc.tile_pool(name="psum", bufs=2, space="PSUM") as pgp,
            tc.tile_pool(name="psum_h", bufs=4, space="PSUM") as php,
            tc.tile_pool(name="psum_t", bufs=2, space="PSUM") as ptp,
        ):
            # PSUM budget (8 banks): pgp tag "mm" [128,512] x3 + php tag
            # "ph" [64,512] x3 + ptp tag "pt" [128,128] x2 = 8.
            nc.gpsimd.load_library(library_config.mlp)
            # ======== constants to SBUF ========
            ident = wp.tile([128, 128], BF16, tag="ident")
            nc.scalar.dma_start(out=ident[:], in_=ident_c[:])
            iota = wp.tile([BL, T], F32, tag="iota")
            nc.scalar.dma_start(out=iota[:], in_=iota_c[:])
            ones_col = wp.tile([BL, 1], F32, tag="ones_col")
            nc.scalar.dma_start(out=ones_col[:], in_=ones_col_c[:])
            iota2c = wp.tile([128, M // 2], F32, tag="iota2c")
            nc.scalar.dma_start(out=iota2c[:], in_=iota2c_c[:])
            pairsel = wp.tile([128, BL], BF16, tag="pairsel")
            nc.scalar.dma_start(out=pairsel[:], in_=pairsel_c[:])

            # ======== weight loads ========
            # x and W_ih are prologue-critical: f32 HWDGE load + DVE cast +
            # PE transpose (no DRAM round-trip). W_hh/W_out bounce through a
            # bf16 DRAM scratch and xbar-transpose in (hidden under gi).
            whh_bf = nc.dram_tensor("whh_bf", [G, V], BF16)
            wout_bf = nc.dram_tensor("wout_bf", [V, 4 * E], BF16)
            nc.gpsimd.dma_start(out=whh_bf[:], in_=W_hh[:])
            nc.gpsimd.dma_start(out=wout_bf[:], in_=W_out[:])

            wi = [wp.tile([128, G], BF16, tag=f"wi{c}", name=f"wi{c}") for c in range(4)]
            wh = [wp.tile([128, G], BF16, tag=f"wh{c}", name=f"wh{c}") for c in range(4)]
            xt = [wp.tile([128, T * BL], BF16, tag=f"xt{c}", name=f"xt{c}") for c in range(4)]
            wo0 = wp.tile([128, V], BF16, tag="wo0")
            wo1 = wp.tile([128, V], BF16, tag="wo1")

            def emit_load_T(src_dram, n_rows, dst_chunks, n_fchunks, tmp_tag):
                for k in range(n_rows):
                    bf = kp.tile([128, n_fchunks * 128], F32, tag=f"{tmp_tag}f",
                                 name=f"{tmp_tag}f{k}")
                    nc.sync.dma_start(
                        out=bf[:], in_=src_dram[128 * k : 128 * (k + 1), :]
                    )
                    bb_ = kp.tile([128, n_fchunks * 128], BF16, tag=f"{tmp_tag}b",
                                  name=f"{tmp_tag}b{k}")
                    nc.vector.tensor_copy(out=bb_[:], in_=bf[:])
                    ptw = ptp.tile([128, n_fchunks * 128], BF16, tag="pt",
                                   name=f"pt{tmp_tag}{k}")
                    for c in range(n_fchunks):
                        nc.tensor.transpose(
                            out=ptw[:, 128 * c : 128 * (c + 1)],
                            in_=bb_[:, 128 * c : 128 * (c + 1)],
                            identity=ident[:],
                        )
                    for c in range(n_fchunks):
                        nc.vector.tensor_copy(
                            out=dst_chunks[c][:, 128 * k : 128 * (k + 1)],
                            in_=ptw[:, 128 * c : 128 * (c + 1)],
                        )

            emit_load_T(med_hist[:].rearrange("t b v -> (t b) v"), 4, xt, 4, "xld")
            emit_load_T(W_ih[:], 12, wi, 4, "wld")

            xbar_insts = []
            for c in range(4):
                xbar_insts.append(nc.sync.dma_start(
                    out=wh[c][:], in_=whh_bf[:, 128 * c : 128 * (c + 1)], transpose=True
                ))
            xbar_insts.append(
                nc.sync.dma_start(out=wo0[:], in_=wout_bf[:, 0:128], transpose=True)
            )
            xbar_insts.append(
                nc.sync.dma_start(out=wo1[:], in_=wout_bf[:, 128:256], transpose=True)
            )
            hwdge_late = []  # plain HWDGE copies to push after the last xbar

            # ======== hist-len mask ========
            hl_i = gp.tile([BL, 1], I32, tag="hl_i")
            nc.scalar.dma_start(out=hl_i[:], in_=hist_len[:])
            hl_f = gp.tile([BL, 1], F32, tag="hl_f")
            nc.vector.tensor_copy(out=hl_f[:], in_=hl_i[:])
            tmask = wp.tile([BL, T], F32, tag="tmask")
            nc.vector.tensor_scalar(
                out=tmask[:], in0=iota[:], scalar1=hl_f[:], scalar2=None, op0=ALU.is_lt
            )
            tmask_bf = wp.tile([BL, T], BF16, tag="tmask_bf")
            nc.vector.tensor_copy(out=tmask_bf[:], in_=tmask[:])

            # pool-section compute must not head-of-line block the PE/DVE
            # streams mid-recurrence: push it after the final h update
            _pool_sec_end = len(nc.cur_bb.bb.instructions)
            if last_h[0] is not None:
                _sec = list(nc.cur_bb.bb.instructions)[_pool_sec_start:_pool_sec_end]
                for _inst in _sec:
                    eng = str(_inst.engine)
                    if eng.endswith("PE") or eng.endswith("DVE"):
                        add_dep_helper(
                            _inst, last_h[0].ins, reason="defer pool compute past GRU"
                        )
            for d in hwdge_late:
                add_dep_helper(d.ins, xbar_insts[-1].ins, reason="defer HWDGE copy past xbars")

            # ======== ddi_adj -> bf16 (one cast-DMA) ========
            A_bf = wp.tile([128, 4 * V], BF16, tag="abf")
            nc.gpsimd.dma_start(
                out=A_bf[:], in_=ddi_adj[:].rearrange("(k p) v -> p k v", p=128)
            )

            # ======== biases ========
            if use_biases:
                ones_row = wp.tile([1, 128], BF16, tag="ones_row")
                nc.scalar.dma_start(out=ones_row[:], in_=ones_row_c[:])
                bo_bf = wp.tile([1, V], BF16, tag="bo_bf")
                nc.gpsimd.dma_start(out=bo_bf[:], in_=b_out[:])
                # gi bias = b_ih + rz_mask * b_hh ; n-part of b_hh added per step
                bi_f = wp.tile([1, G], F32, tag="bi_f")
                nc.scalar.dma_start(out=bi_f[:], in_=b_ih[:])
                bh_f = wp.tile([1, G], F32, tag="bh_f")
                nc.scalar.dma_start(out=bh_f[:], in_=b_hh[:])
                rz_m = wp.tile([1, G], F32, tag="rz_m")
                nc.scalar.dma_start(out=rz_m[:], in_=rz_mask_c[:])
                gi_bias = wp.tile([1, G], F32, tag="gi_bias")
                nc.vector.tensor_tensor(
                    out=gi_bias[:], in0=bh_f[:], in1=rz_m[:], op=ALU.mult
                )
                nc.vector.tensor_tensor(
                    out=gi_bias[:], in0=gi_bias[:], in1=bi_f[:], op=ALU.add
                )
                gi_bias_bf = wp.tile([1, G], BF16, tag="gi_bias_bf")
                nc.vector.tensor_copy(out=gi_bias_bf[:], in_=gi_bias[:])
                bh_n_bf = wp.tile([1, V], BF16, tag="bh_n_bf")
                nc.vector.tensor_copy(out=bh_n_bf[:], in_=bh_f[:, 2 * V : 3 * V])
                # materialize b_hh_n broadcast to all samples (K=1 matmul)
                pbh = pgp.tile([BL, V], F32, tag="mm")
                nc.tensor.matmul(
                    out=pbh[:], lhsT=ones_row[:, :BL], rhs=bh_n_bf[:],
                    start=True, stop=True,
                )
                bhn_mat = wp.tile([BL, V], BF16, tag="bhn_mat")
                nc.vector.tensor_copy(out=bhn_mat[:], in_=pbh[:])

            # ======== gi = x @ W_ih.T (+ folded biases), all 8 steps ========
            gi_sb = [wp.tile([128, G], BF16, tag=f"gi{j}", name=f"gi{j}") for j in range(4)]
            # odd steps live at partition base 64 inside gi_sb; compute
            # engines need base-0 operands, so DMA the odd halves down


            def emit_gi_tile(j):
                for ns in range(3):
                    pg = pgp.tile([128, 512], F32, tag="mm")
                    for c in range(4):
                        nc.tensor.matmul(
                            out=pg[:],
                            lhsT=xt[c][:, 128 * j : 128 * (j + 1)],
                            rhs=wi[c][:, 512 * ns : 512 * (ns + 1)],
                            start=(c == 0),
                            stop=(c == 3) if not use_biases else False,
                        )
                    if use_biases:
                        nc.tensor.matmul(
                            out=pg[:],
                            lhsT=ones_row[:, :128],
                            rhs=gi_bias_bf[:, 512 * ns : 512 * (ns + 1)],
                            start=False,
                            stop=True,
                        )
                    nc.scalar.activation(
                        out=gi_sb[j][:, 512 * ns : 512 * (ns + 1)], in_=pg[:],
                        func=AF.Copy,
                    )

            # ======== GRU recurrence ========
            gi_odd = [
                wp.tile([BL, V], BF16, tag=f"giodd{j}", name=f"giodd{j}")
                for j in range(4)
            ]
            h_sb = wp.tile([BL, V], BF16, tag="h_sb")
            hT = wp.tile([128, 4 * BL], BF16, tag="hT")

            last_h = [None]

            def gate_slices(t):
                """gi slices for step t; odd steps live at partition base 64
                and are only consumed by base-64 identity matmuls."""
                j, half = t // 2, (t % 2) * BL
                sl = gi_sb[j][half : half + BL, :]
                return sl[:, 0:V], sl[:, V : 2 * V], sl[:, 2 * V : 3 * V]

            def emit_hT(t):
                ptt = ptp.tile([128, 4 * BL], BF16, tag="pt", name=f"ptt{t}")
                for c in range(4):
                    nc.tensor.transpose(
                        out=ptt[:, BL * c : BL * (c + 1)],
                        in_=h_sb[:, 128 * c : 128 * (c + 1)],
                        identity=ident[:BL, :BL],
                    )
                nc.vector.tensor_copy(out=hT[:], in_=ptt[:])

            def _gi_odd_dma(j):
                hwdge_late.append(nc.sync.dma_start(
                    out=gi_odd[j][:], in_=gi_sb[j][BL:128, 2 * V : 3 * V]
                ))

            def emit_step0():
                gi_r, gi_z, gi_n = gate_slices(0)
                zp = kp.tile([BL, V], BF16, tag="zp")
                nc.scalar.activation(out=zp[:], in_=gi_z, func=AF.Sigmoid, scale=-1.0)
                n_t = kp.tile([BL, V], BF16, tag="n_t")
                if use_biases:
                    # n = tanh(gi_n + sigmoid(gi_r) * b_hh_n)   (h0 = 0)
                    r_t = kp.tile([BL, V], BF16, tag="r_t")
                    nc.scalar.activation(out=r_t[:], in_=gi_r, func=AF.Sigmoid)
                    p_t = kp.tile([BL, V], BF16, tag="p_t")
                    nc.vector.tensor_tensor(
                        out=p_t[:], in0=r_t[:], in1=bhn_mat[:], op=ALU.mult
                    )
                    a_n = kp.tile([BL, V], BF16, tag="a_n")
                    nc.vector.tensor_tensor(out=a_n[:], in0=p_t[:], in1=gi_n, op=ALU.add)
                    nc.scalar.activation(out=n_t[:], in_=a_n[:], func=AF.Tanh)
                else:
                    nc.scalar.activation(out=n_t[:], in_=gi_n, func=AF.Tanh)
                # h = m * z' * n
                w_t = kp.tile([BL, V], BF16, tag="w_t")
                nc.vector.tensor_tensor(
                    out=w_t[:], in0=zp[:],
                    in1=tmask_bf[:, 0:1].to_broadcast([BL, V]), op=ALU.mult,
                )
                nc.vector.tensor_tensor(out=h_sb[:], in0=w_t[:], in1=n_t[:], op=ALU.mult)
                emit_hT(0)

            def emit_step(t):
                gi_r, gi_z, gi_n = gate_slices(t)
                # gh psum slices, n first (its consumer chain is longest)
                ph_n = php.tile([BL, 512], F32, tag="ph")
                ph_r = php.tile([BL, 512], F32, tag="ph")
                ph_z = php.tile([BL, 512], F32, tag="ph")
                # gh psum accumulations; the r/z slices also fold the gi add
                # in via an extra K=64 identity matmul so ACT can consume the
                # PSUM directly (saves a DVE add + a hop per gate)
                for ns, ph, gi_sl in ((2, ph_n, None), (0, ph_r, gi_r), (1, ph_z, gi_z)):
                    for c in range(4):
                        nc.tensor.matmul(
                            out=ph[:],
                            lhsT=hT[:, BL * c : BL * (c + 1)],
                            rhs=wh[c][:, 512 * ns : 512 * (ns + 1)],
                            start=(c == 0),
                            stop=(c == 3) and gi_sl is None,
                        )
                    if gi_sl is not None:
                        idnt = (
                            ident[:BL, :BL]
                            if t % 2 == 0
                            else ident[BL:128, BL:128]
                        )
                        nc.tensor.matmul(
                            out=ph[:], lhsT=idnt, rhs=gi_sl,
                            start=False, stop=True,
                        )
                r_t = kp.tile([BL, V], BF16, tag="r_t")
                nc.scalar.activation(out=r_t[:], in_=ph_r[:], func=AF.Sigmoid)
                zp = kp.tile([BL, V], BF16, tag="zp")  # z' = 1-z = sigmoid(-a_z)
                nc.scalar.activation(out=zp[:], in_=ph_z[:], func=AF.Sigmoid, scale=-1.0)
                w_t = kp.tile([BL, V], BF16, tag="w_t")  # w = m * z'
                nc.vector.tensor_tensor(
                    out=w_t[:], in0=zp[:],
                    in1=tmask_bf[:, t : t + 1].to_broadcast([BL, V]), op=ALU.mult,
                )
                p_t = kp.tile([BL, V], BF16, tag="p_t")
                if use_biases:
                    ghn_bf = kp.tile([BL, V], BF16, tag="ghn_bf")
                    nc.vector.tensor_tensor(
                        out=ghn_bf[:], in0=ph_n[:], in1=bhn_mat[:], op=ALU.add
                    )
                    nc.vector.tensor_tensor(
                        out=p_t[:], in0=r_t[:], in1=ghn_bf[:], op=ALU.mult
                    )
                else:
                    nc.vector.tensor_tensor(
                        out=p_t[:], in0=ph_n[:], in1=r_t[:], op=ALU.mult
                    )
                gi_n_b0 = gi_n if t % 2 == 0 else gi_odd[t // 2][:]
                a_n = kp.tile([BL, V], BF16, tag="a_n")
                nc.vector.tensor_tensor(out=a_n[:], in0=p_t[:], in1=gi_n_b0, op=ALU.add)
                n_t = kp.tile([BL, V], BF16, tag="n_t")
                nc.scalar.activation(out=n_t[:], in_=a_n[:], func=AF.Tanh)
                u_t = kp.tile([BL, V], BF16, tag="u_t")
                nc.vector.tensor_tensor(out=u_t[:], in0=n_t[:], in1=h_sb[:], op=ALU.subtract)
                t1 = kp.tile([BL, V], BF16, tag="t1")
                nc.vector.tensor_tensor(out=t1[:], in0=w_t[:], in1=u_t[:], op=ALU.mult)
                last_h[0] = nc.vector.tensor_tensor(
                    out=h_sb[:], in0=h_sb[:], in1=t1[:], op=ALU.add
                )
                if t < T - 1:
                    emit_hT(t)

            # interleave gi tiles with early steps so PE stays busy during
            # the gate chains
            emit_gi_tile(0)
            _gi_odd_dma(0)
            if debug:
                nc.gpsimd.dma_start(out=dbg_ext["dbg_gi0"][:], in_=gi_sb[0][:])
                nc.gpsimd.dma_start(out=dbg_ext["dbg_wi0"][:], in_=wi[0][:])
                nc.gpsimd.dma_start(out=dbg_ext["dbg_xt0"][:], in_=xt[0][:])
            emit_step0()
            if debug:
                nc.gpsimd.dma_start(out=dbg_ext["dbg_h0"][:], in_=h_sb[:])
            emit_gi_tile(1)
            _gi_odd_dma(1)
            emit_step(1)
            if debug:
                nc.gpsimd.dma_start(out=dbg_ext["dbg_h1"][:], in_=h_sb[:])
            emit_gi_tile(2)
            _gi_odd_dma(2)
            emit_step(2)
            emit_gi_tile(3)
            _gi_odd_dma(3)
            for t in range(3, T):
                emit_step(t)

            if debug:
                nc.gpsimd.dma_start(out=dbg_ext["dbg_hF"][:], in_=h_sb[:])
            # ======== logits + out ========
            pl = pgp.tile([BL, V], F32, tag="mm")
            nc.tensor.matmul(
                out=pl[:], lhsT=finT0[:], rhs=wo0[:], start=True, stop=False
            )
            nc.tensor.matmul(
                out=pl[:],
                lhsT=finT1[:],
                rhs=wo1[:],
                start=False,
                stop=not use_biases,
            )
            if use_biases:
                nc.tensor.matmul(
                    out=pl[:], lhsT=ones_row[:, :BL], rhs=bo_bf[:], start=False,
                    stop=True,
                )
            out_f = wp.tile([BL, V], F32, tag="out_f")
            nc.vector.tensor_tensor(out=out_f[:], in0=pl[:], in1=h_sb[:], op=ALU.add)
            nc.sync.dma_start(out=out_ext[:], in_=out_f[:])

            # ======== DDI score partial ========
            s_bf = wp.tile([BL, V], BF16, tag="s_bf")
            nc.scalar.activation(out=s_bf[:], in_=out_f[:], func=AF.Sigmoid)
            sT = wp.tile([128, 4 * BL], BF16, tag="sT")
            pts = ptp.tile([128, 4 * BL], BF16, tag="pt", name="pts")
            for c in range(4):
                nc.tensor.transpose(
                    out=pts[:, BL * c : BL * (c + 1)],
                    in_=s_bf[:, 128 * c : 128 * (c + 1)],
                    identity=ident[:BL, :BL],
                )
            nc.vector.tensor_copy(out=sT[:], in_=pts[:])
            pq = pgp.tile([BL, V], F32, tag="mm")
            for c in range(4):
                nc.tensor.matmul(
                    out=pq[:],
                    lhsT=sT[:, BL * c : BL * (c + 1)],
                    rhs=A_bf[:, V * c : V * (c + 1)],
                    start=(c == 0),
                    stop=(c == 3),
                )
            sq = kp.tile([BL, V], F32, tag="sq")
            nc.vector.tensor_tensor(out=sq[:], in0=pq[:], in1=s_bf[:], op=ALU.mult)
            rcol = kp.tile([BL, 1], F32, tag="rcol")
            nc.vector.tensor_reduce(
                out=rcol[:], in_=sq[:], axis=mybir.AxisListType.X, op=ALU.add
            )
            psc = ptp.tile([1, 1], F32, tag="pt")
            nc.tensor.matmul(
                out=psc[:], lhsT=rcol[:], rhs=ones_col[:], start=True, stop=True
            )
            sc_sb = kp.tile([1, 1], F32, tag="sc_sb")
            nc.vector.tensor_copy(out=sc_sb[:], in_=psc[:])
            nc.sync.dma_start(out=ddi_ext[:], in_=sc_sb[:])

    split_waits(nc, limit=1)
    from concourse.library_overlay import lower_extended_insts

    lower_extended_insts(nc)
    return nc


def split_waits(nc, limit=1):
    """walrus in this toolchain only accepts `limit` sem-waits per
    instruction; move excess waits onto same-engine nops placed before."""
    for f in nc.m.functions:
        for bb in f.blocks:
            insts = list(bb.instructions)
            out = []
            for inst in insts:
                si = inst.sync_info
                waits = list(si.on_wait) if si and si.on_wait else []
                if len(waits) > limit:
                    extra, keep = waits[:-limit], waits[-limit:]
                    for w in extra:
                        nop = nc.engines[inst.engine].nop(nofuse=True).ins
                        for f2 in nc.m.functions:
                            for bb2 in f2.blocks:
                                if nop in list(bb2.instructions):
                                    bb2.instructions.remove(nop)
                        nop.sync_info = mybir.SyncInfo(on_wait=[w], on_update=[])
                        out.append(nop)
                    si.on_wait = keep
                out.append(inst)
            bb.instructions[:] = out


def make_in_maps(inputs):
    """Shard the full inputs into per-core input maps."""
    diagnose = np.ascontiguousarray(np.asarray(inputs["diagnose"]).astype(np.int32))
    procedures = np.ascontiguousarray(np.asarray(inputs["procedures"]).astype(np.int32))
    last_meds = np.ascontiguousarray(np.asarray(inputs["last_meds"]).astype(np.int32))

    def wrap16(a):
        """[BL, S] ints -> dma_gather wrapped idx list [16, BL*S/16] i16,
        flat order i = b + BL*s (column-major), wrapped i -> (i%16, i//16)."""
        flat = a.T.reshape(-1).astype(np.int16)
        return np.ascontiguousarray(flat.reshape(-1, 16).T)
    med_hist = np.ascontiguousarray(np.asarray(inputs["med_hist"], np.float32))
    hist_len = np.ascontiguousarray(
        np.asarray(inputs["hist_len"]).astype(np.int32).reshape(B, 1)
    )
    ddi_adj = np.ascontiguousarray(np.asarray(inputs["ddi_adj"], np.float32))
    diag_table = np.ascontiguousarray(np.asarray(inputs["diag_table"], np.float32))
    proc_table = np.ascontiguousarray(np.asarray(inputs["proc_table"], np.float32))
    med_table = np.ascontiguousarray(np.asarray(inputs["med_table"], np.float32))
    W_out = np.asarray(inputs["W_out"], np.float32)
    W_out = np.ascontiguousarray(
        np.concatenate([W_out, np.zeros((V, E), np.float32)], axis=1)
    )
    b_out = np.ascontiguousarray(np.asarray(inputs["b_out"], np.float32).reshape(1, V))
    W_ih = np.ascontiguousarray(np.asarray(inputs["W_ih"], np.float32))
    W_hh = np.ascontiguousarray(np.asarray(inputs["W_hh"], np.float32))
    b_ih = np.ascontiguousarray(np.asarray(inputs["b_ih"], np.float32).reshape(1, G))
    b_hh = np.ascontiguousarray(np.asarray(inputs["b_hh"], np.float32).reshape(1, G))

    in_maps = []
    for k in range(N_CORES):
        sl = slice(k * BL, (k + 1) * BL)
        in_maps.append(
            dict(
                diag_idx16=wrap16(diagnose[sl]),
                proc_idx16=wrap16(procedures[sl]),
                med_idx16=wrap16(last_meds[sl]),
                last_meds=last_meds[sl],
                med_hist=np.ascontiguousarray(med_hist[sl].transpose(1, 0, 2)),
                hist_len=hist_len[sl],
                ddi_adj=ddi_adj,
                diag_table=diag_table,
                proc_table=proc_table,
                med_table=med_table,
                W_out=W_out,
                b_out=b_out,
                W_ih=W_ih,
                W_hh=W_hh,
                b_ih=b_ih,
                b_hh=b_hh,
            )
        )
    use_biases = bool(
        np.any(b_out) or np.any(b_ih) or np.any(b_hh)
    )
    return in_maps, use_biases


_NC_CACHE = {}


def kernel(**inputs):
    from concourse.bass_utils import run_bass_kernel_spmd

    in_maps, use_biases = make_in_maps(inputs)
    if use_biases not in _NC_CACHE:
        _NC_CACHE[use_biases] = build(use_biases)
    nc = _NC_CACHE[use_biases]
    res = run_bass_kernel_spmd(nc, in_maps, core_ids=list(range(N_CORES)))
    out = np.concatenate([res.results[k]["out"] for k in range(N_CORES)], axis=0)
    total = float(sum(float(res.results[k]["ddi"][0, 0]) for k in range(N_CORES)))
    score = np.float32(KGLOSS_SCALE * total / B)
    return out.astype(np.float32), score


# revision 46
# speedup vs baseline: 1.7116x; 1.0204x over previous
"""Trainium2 Bass kernel for nn_AIModel_34892314312864 (ragged_sequence).

Model (per sample):
  pools  = gather-sum embeddings (diagnose[40], procedures[30], masked last_meds[20])
  logits = concat(pools) @ W_out.T + b_out
  hist   = last hidden of a length-masked GRU over med_hist [T=8, V=512]
  out    = logits + hist
  score  = 0.0005 * mean_b( sigmoid(out)_b @ ddi_adj @ sigmoid(out)_b )

Distribution: pure data parallel, batch 512 sharded 64/core across 8 cores.
Weights replicated. DDI score: per-core partial sums, combined on host.

Key device techniques:
  - Embedding pools: one dma_gather per table with a column-major int16
    index list (sample b's even/odd slots land in partitions b / b+64,
    slots on the free dim), masked strided DVE reduce, one pair-sum
    matmul. The gathers run on the Q7 queue fully hidden under the GRU.
  - All PE compute in bf16. x and W_ih (prologue-critical) load as f32 +
    DVE cast + PE transposes; W_hh/W_out bounce through a bf16 DRAM
    scratch and xbar-transpose in, hidden under the gi matmuls.
  - GRU: gi = x @ W_ih.T precomputed for all 8 steps (M=128 matmuls)
    and interleaved into the recurrence to keep the PE warm; per step
    the gi r/z adds are folded into the gh PSUM accumulation as K=64
    identity matmuls (base-64 identities fix odd-step partition bases),
    so sigmoids read PSUM directly; gates run on ACT, elementwise on
    DVE; h is re-transposed each step via packed PE transposes.
  - Pool/logits/DDI compute is dependency-deferred behind the final h
    update so it cannot head-of-line block the PE/DVE streams.
"""

import sys

sys.path.insert(0, "/opt/trn_rl_repo")

import numpy as np

from concourse import bass, library_config, mybir
from concourse.tile import TileContext, add_dep_helper

F32 = mybir.dt.float32
BF16 = mybir.dt.bfloat16
I32 = mybir.dt.int32
I16 = mybir.dt.int16
U16 = mybir.dt.uint16
AF = mybir.ActivationFunctionType
ALU = mybir.AluOpType

# Problem constants (hardcoded; kernel.py must be self-contained)
B, D, P, M, T, V, E = 512, 40, 30, 20, 8, 512, 64
DV, PV = 2000, 1500
MED_PAD = V + 2  # 514
G = 3 * V  # 1536
N_CORES = 8
BL = B // N_CORES  # 64 samples per core
KGLOSS_SCALE = 0.001 * 0.5

DIAG_ROWS, PROC_ROWS, MED_ROWS = DV + 3, PV + 3, V + 3


def build(use_biases: bool, debug: bool = False):
    nc = bass.Bass()
    dbg_ext = {}
    if debug:
        for nm, shape in [
            ("dbg_fin", [BL, 3 * E]),
            ("dbg_gi0", [128, G]),
            ("dbg_h0", [BL, V]),
            ("dbg_h1", [BL, V]),
            ("dbg_hF", [BL, V]),
            ("dbg_wi0", [128, G]),
            ("dbg_xt0", [128, 512]),
        ]:
            dbg_ext[nm] = nc.declare_dram_parameter(nm, shape, F32, isOutput=True)

    # ---- parameters (per-core shards / replicated weights) ----
    diag_idx16 = nc.declare_dram_parameter("diag_idx16", [16, BL * D // 16], I16, isOutput=False)
    proc_idx16 = nc.declare_dram_parameter("proc_idx16", [16, BL * P // 16], I16, isOutput=False)
    med_idx16 = nc.declare_dram_parameter("med_idx16", [16, BL * M // 16], I16, isOutput=False)
    last_meds = nc.declare_dram_parameter("last_meds", [BL, M], I32, isOutput=False)
    # med_hist is passed t-major ([T, BL, V]) so the bf16 cast-DMA is contiguous
    med_hist = nc.declare_dram_parameter("med_hist", [T, BL, V], F32, isOutput=False)
    hist_len = nc.declare_dram_parameter("hist_len", [BL, 1], I32, isOutput=False)
    ddi_adj = nc.declare_dram_parameter("ddi_adj", [V, V], F32, isOutput=False)
    diag_table = nc.declare_dram_parameter("diag_table", [DIAG_ROWS, E], F32, isOutput=False)
    proc_table = nc.declare_dram_parameter("proc_table", [PROC_ROWS, E], F32, isOutput=False)
    med_table = nc.declare_dram_parameter("med_table", [MED_ROWS, E], F32, isOutput=False)
    # W_out is zero-padded on host from [V, 192] to [V, 256] so its transpose
    # splits into two full 128-row chunks
    W_out = nc.declare_dram_parameter("W_out", [V, 4 * E], F32, isOutput=False)
    b_out = nc.declare_dram_parameter("b_out", [1, V], F32, isOutput=False)
    W_ih = nc.declare_dram_parameter("W_ih", [G, V], F32, isOutput=False)
    W_hh = nc.declare_dram_parameter("W_hh", [G, V], F32, isOutput=False)
    b_ih = nc.declare_dram_parameter("b_ih", [1, G], F32, isOutput=False)
    b_hh = nc.declare_dram_parameter("b_hh", [1, G], F32, isOutput=False)
    out_ext = nc.declare_dram_parameter("out", [BL, V], F32, isOutput=True)
    ddi_ext = nc.declare_dram_parameter("ddi", [1, 1], F32, isOutput=True)

    # ---- inline constants ----
    import ml_dtypes

    ident_np = np.zeros((128, 128), np.float32)
    np.fill_diagonal(ident_np, 1.0)

    ident_c = nc.inline_tensor(ident_np.astype(ml_dtypes.bfloat16), name="ident_bf")
    iota_c = nc.inline_tensor(
        np.tile(np.arange(T, dtype=np.float32), (BL, 1)), name="iota8"
    )
    ones_col_c = nc.inline_tensor(np.ones((BL, 1), np.float32), name="ones_col")
    # iota2c[p, c] = 2c + (p >= 64): the slot number at (partition, chunk)
    iota2c_np = (2 * np.arange(M // 2, dtype=np.float32))[None, :] + (
        np.arange(128) >= BL
    ).astype(np.float32)[:, None]
    iota2c_c = nc.inline_tensor(iota2c_np, name="iota2c")
    # pairsel[p, b] = 1 if p % 64 == b (sums partition b and 64+b)
    pairsel_np = np.zeros((128, BL), np.float32)
    pairsel_np[np.arange(128), np.arange(128) % BL] = 1.0
    pairsel_c = nc.inline_tensor(pairsel_np.astype(ml_dtypes.bfloat16), name="pairsel")
    if use_biases:
        ones_row_c = nc.inline_tensor(
            np.ones((1, 128), ml_dtypes.bfloat16), name="ones_row"
        )
        # mask that keeps r,z parts of b_hh and zeroes the n part
        rz_np = np.zeros((1, G), np.float32)
        rz_np[:, : 2 * V] = 1.0
        rz_mask_c = nc.inline_tensor(rz_np, name="rz_mask")

    with TileContext(nc) as tc:
        with (
            tc.tile_pool(name="wts", bufs=1) as wp,
            tc.tile_pool(name="work", bufs=2) as kp,
            tc.tile_pool(name="gath", bufs=1) as gp,
            tc.tile_pool(name="psum", bufs=2, space="PSUM") as pgp,
            tc.tile_pool(name="psum_h", bufs=4, space="PSUM") as php,
            tc.tile_pool(name="psum_t", bufs=2, space="PSUM") as ptp,
        ):
            # PSUM budget (8 banks): pgp tag "mm" [128,512] x3 + php tag
            # "ph" [64,512] x3 + ptp tag "pt" [128,128] x2 = 8.
            nc.gpsimd.load_library(library_config.mlp)
            # ======== constants to SBUF ========
            ident = wp.tile([128, 128], BF16, tag="ident")
            nc.scalar.dma_start(out=ident[:], in_=ident_c[:])
            iota = wp.tile([BL, T], F32, tag="iota")
            nc.scalar.dma_start(out=iota[:], in_=iota_c[:])
            ones_col = wp.tile([BL, 1], F32, tag="ones_col")
            nc.scalar.dma_start(out=ones_col[:], in_=ones_col_c[:])
            iota2c = wp.tile([128, M // 2], F32, tag="iota2c")
            nc.scalar.dma_start(out=iota2c[:], in_=iota2c_c[:])
            pairsel = wp.tile([128, BL], BF16, tag="pairsel")
            nc.scalar.dma_start(out=pairsel[:], in_=pairsel_c[:])

            # ======== weight loads ========
            # x and W_ih are prologue-critical: f32 HWDGE load + DVE cast +
            # PE transpose (no DRAM round-trip). W_hh/W_out bounce through a
            # bf16 DRAM scratch and xbar-transpose in (hidden under gi).
            whh_bf = nc.dram_tensor("whh_bf", [G, V], BF16)
            wout_bf = nc.dram_tensor("wout_bf", [V, 4 * E], BF16)
            nc.gpsimd.dma_start(out=whh_bf[:], in_=W_hh[:])
            nc.gpsimd.dma_start(out=wout_bf[:], in_=W_out[:])

            wi = [wp.tile([128, G], BF16, tag=f"wi{c}", name=f"wi{c}") for c in range(4)]
            wh = [wp.tile([128, G], BF16, tag=f"wh{c}", name=f"wh{c}") for c in range(4)]
            xt = [wp.tile([128, T * BL], BF16, tag=f"xt{c}", name=f"xt{c}") for c in range(4)]
            wo0 = wp.tile([128, V], BF16, tag="wo0")
            wo1 = wp.tile([128, V], BF16, tag="wo1")

            def emit_load_T(src_dram, n_rows, dst_chunks, n_fchunks, tmp_tag):
                for k in range(n_rows):
                    bf = kp.tile([128, n_fchunks * 128], F32, tag=f"{tmp_tag}f",
                                 name=f"{tmp_tag}f{k}")
                    nc.sync.dma_start(
                        out=bf[:], in_=src_dram[128 * k : 128 * (k + 1), :]
                    )
                    bb_ = kp.tile([128, n_fchunks * 128], BF16, tag=f"{tmp_tag}b",
                                  name=f"{tmp_tag}b{k}")
                    nc.vector.tensor_copy(out=bb_[:], in_=bf[:])
                    ptw = ptp.tile([128, n_fchunks * 128], BF16, tag="pt",
                                   name=f"pt{tmp_tag}{k}")
                    for c in range(n_fchunks):
                        nc.tensor.transpose(
                            out=ptw[:, 128 * c : 128 * (c + 1)],
                            in_=bb_[:, 128 * c : 128 * (c + 1)],
                            identity=ident[:],
                        )
                    for c in range(n_fchunks):
                        nc.vector.tensor_copy(
                            out=dst_chunks[c][:, 128 * k : 128 * (k + 1)],
                            in_=ptw[:, 128 * c : 128 * (c + 1)],
                        )

            emit_load_T(med_hist[:].rearrange("t b v -> (t b) v"), 4, xt, 4, "xld")
            emit_load_T(W_ih[:], 12, wi, 4, "wld")

            xbar_insts = []
            for c in range(4):
                xbar_insts.append(nc.sync.dma_start(
                    out=wh[c][:], in_=whh_bf[:, 128 * c : 128 * (c + 1)], transpose=True
                ))
            xbar_insts.append(
                nc.sync.dma_start(out=wo0[:], in_=wout_bf[:, 0:128], transpose=True)
            )
            xbar_insts.append(
                nc.sync.dma_start(out=wo1[:], in_=wout_bf[:, 128:256], transpose=True)
            )
            hwdge_late = []  # plain HWDGE copies to push after the last xbar

            # ======== hist-len mask ========
            hl_i = gp.tile([BL, 1], I32, tag="hl_i")
            nc.scalar.dma_start(out=hl_i[:], in_=hist_len[:])
            hl_f = gp.tile([BL, 1], F32, tag="hl_f")
            nc.vector.tensor_copy(out=hl_f[:], in_=hl_i[:])
            tmask = wp.tile([BL, T], F32, tag="tmask")
            nc.vector.tensor_scalar(
                out=tmask[:], in0=iota[:], scalar1=hl_f[:], scalar2=None, op0=ALU.is_lt
            )
            tmask_bf = wp.tile([BL, T], BF16, tag="tmask_bf")
            nc.vector.tensor_copy(out=tmask_bf[:], in_=tmask[:])

            # pool-section compute must not head-of-line block the PE/DVE
            # streams mid-recurrence: push it after the final h update
            _pool_sec_end = len(nc.cur_bb.bb.instructions)
            if last_h[0] is not None:
                _sec = list(nc.cur_bb.bb.instructions)[_pool_sec_start:_pool_sec_end]
                for _inst in _sec:
                    eng = str(_inst.engine)
                    if eng.endswith("PE") or eng.endswith("DVE"):
                        add_dep_helper(
                            _inst, last_h[0].ins, reason="defer pool compute past GRU"
                        )
            for d in hwdge_late:
                add_dep_helper(d.ins, xbar_insts[-1].ins, reason="defer HWDGE copy past xbars")

            # ======== ddi_adj -> bf16 (one cast-DMA) ========
            A_bf = wp.tile([128, 4 * V], BF16, tag="abf")
            nc.gpsimd.dma_start(
                out=A_bf[:], in_=ddi_adj[:].rearrange("(k p) v -> p k v", p=128)
            )

            # ======== biases ========
            if use_biases:
                ones_row = wp.tile([1, 128], BF16, tag="ones_row")
                nc.scalar.dma_start(out=ones_row[:], in_=ones_row_c[:])
                bo_bf = wp.tile([1, V], BF16, tag="bo_bf")
                nc.gpsimd.dma_start(out=bo_bf[:], in_=b_out[:])
                # gi bias = b_ih + rz_mask * b_hh ; n-part of b_hh added per step
                bi_f = wp.tile([1, G], F32, tag="bi_f")
                nc.scalar.dma_start(out=bi_f[:], in_=b_ih[:])
                bh_f = wp.tile([1, G], F32, tag="bh_f")
                nc.scalar.dma_start(out=bh_f[:], in_=b_hh[:])
                rz_m = wp.tile([1, G], F32, tag="rz_m")
                nc.scalar.dma_start(out=rz_m[:], in_=rz_mask_c[:])
                gi_bias = wp.tile([1, G], F32, tag="gi_bias")
                nc.vector.tensor_tensor(
                    out=gi_bias[:], in0=bh_f[:], in1=rz_m[:], op=ALU.mult
                )
                nc.vector.tensor_tensor(
                    out=gi_bias[:], in0=gi_bias[:], in1=bi_f[:], op=ALU.add
                )
                gi_bias_bf = wp.tile([1, G], BF16, tag="gi_bias_bf")
                nc.vector.tensor_copy(out=gi_bias_bf[:], in_=gi_bias[:])
                bh_n_bf = wp.tile([1, V], BF16, tag="bh_n_bf")
                nc.vector.tensor_copy(out=bh_n_bf[:], in_=bh_f[:, 2 * V : 3 * V])
                # materialize b_hh_n broadcast to all samples (K=1 matmul)
                pbh = pgp.tile([BL, V], F32, tag="mm")
                nc.tensor.matmul(
                    out=pbh[:], lhsT=ones_row[:, :BL], rhs=bh_n_bf[:],
                    start=True, stop=True,
                )
                bhn_mat = wp.tile([BL, V], BF16, tag="bhn_mat")
                nc.vector.tensor_copy(out=bhn_mat[:], in_=pbh[:])

            # ======== gi = x @ W_ih.T (+ folded biases), all 8 steps ========
            gi_sb = [wp.tile([128, G], BF16, tag=f"gi{j}", name=f"gi{j}") for j in range(4)]
            # odd steps live at partition base 64 inside gi_sb; compute
            # engines need base-0 operands, so DMA the odd halves down


            def emit_gi_tile(j):
                for ns in range(3):
                    pg = pgp.tile([128, 512], F32, tag="mm")
                    for c in range(4):
                        nc.tensor.matmul(
                            out=pg[:],
                            lhsT=xt[c][:, 128 * j : 128 * (j + 1)],
                            rhs=wi[c][:, 512 * ns : 512 * (ns + 1)],
                            start=(c == 0),
                            stop=(c == 3) if not use_biases else False,
                        )
                    if use_biases:
                        nc.tensor.matmul(
                            out=pg[:],
                            lhsT=ones_row[:, :128],
                            rhs=gi_bias_bf[:, 512 * ns : 512 * (ns + 1)],
                            start=False,
                            stop=True,
                        )
                    nc.scalar.activation(
                        out=gi_sb[j][:, 512 * ns : 512 * (ns + 1)], in_=pg[:],
                        func=AF.Copy,
                    )

            # ======== GRU recurrence ========
            gi_odd = [
                wp.tile([BL, V], BF16, tag=f"giodd{j}", name=f"giodd{j}")
                for j in range(4)
            ]
            h_sb = wp.tile([BL, V], BF16, tag="h_sb")
            hT = wp.tile([128, 4 * BL], BF16, tag="hT")

            last_h = [None]

            def gate_slices(t):
                """gi slices for step t; odd steps live at partition base 64
                and are only consumed by base-64 identity matmuls."""
                j, half = t // 2, (t % 2) * BL
                sl = gi_sb[j][half : half + BL, :]
                return sl[:, 0:V], sl[:, V : 2 * V], sl[:, 2 * V : 3 * V]

            def emit_hT(t):
                ptt = ptp.tile([128, 4 * BL], BF16, tag="pt", name=f"ptt{t}")
                for c in range(4):
                    nc.tensor.transpose(
                        out=ptt[:, BL * c : BL * (c + 1)],
                        in_=h_sb[:, 128 * c : 128 * (c + 1)],
                        identity=ident[:BL, :BL],
                    )
                nc.vector.tensor_copy(out=hT[:], in_=ptt[:])

            def _gi_odd_dma(j):
                hwdge_late.append(nc.sync.dma_start(
                    out=gi_odd[j][:], in_=gi_sb[j][BL:128, 2 * V : 3 * V]
                ))

            def emit_step0():
                gi_r, gi_z, gi_n = gate_slices(0)
                zp = kp.tile([BL, V], BF16, tag="zp")
                nc.scalar.activation(out=zp[:], in_=gi_z, func=AF.Sigmoid, scale=-1.0)
                n_t = kp.tile([BL, V], BF16, tag="n_t")
                if use_biases:
                    # n = tanh(gi_n + sigmoid(gi_r) * b_hh_n)   (h0 = 0)
                    r_t = kp.tile([BL, V], BF16, tag="r_t")
                    nc.scalar.activation(out=r_t[:], in_=gi_r, func=AF.Sigmoid)
                    p_t = kp.tile([BL, V], BF16, tag="p_t")
                    nc.vector.tensor_tensor(
                        out=p_t[:], in0=r_t[:], in1=bhn_mat[:], op=ALU.mult
                    )
                    a_n = kp.tile([BL, V], BF16, tag="a_n")
                    nc.vector.tensor_tensor(out=a_n[:], in0=p_t[:], in1=gi_n, op=ALU.add)
                    nc.scalar.activation(out=n_t[:], in_=a_n[:], func=AF.Tanh)
                else:
                    nc.scalar.activation(out=n_t[:], in_=gi_n, func=AF.Tanh)
                # h = m * z' * n
                w_t = kp.tile([BL, V], BF16, tag="w_t")
                nc.vector.tensor_tensor(
                    out=w_t[:], in0=zp[:],
                    in1=tmask_bf[:, 0:1].to_broadcast([BL, V]), op=ALU.mult,
                )
                nc.vector.tensor_tensor(out=h_sb[:], in0=w_t[:], in1=n_t[:], op=ALU.mult)
                emit_hT(0)

            def emit_step(t):
                gi_r, gi_z, gi_n = gate_slices(t)
                # gh psum slices, n first (its consumer chain is longest)
                ph_n = php.tile([BL, 512], F32, tag="ph")
                ph_r = php.tile([BL, 512], F32, tag="ph")
                ph_z = php.tile([BL, 512], F32, tag="ph")
                # gh psum accumulations; the r/z slices also fold the gi add
                # in via an extra K=64 identity matmul so ACT can consume the
                # PSUM directly (saves a DVE add + a hop per gate)
                for ns, ph, gi_sl in ((0, ph_r, gi_r), (2, ph_n, None), (1, ph_z, gi_z)):
                    for c in range(4):
                        nc.tensor.matmul(
                            out=ph[:],
                            lhsT=hT[:, BL * c : BL * (c + 1)],
                            rhs=wh[c][:, 512 * ns : 512 * (ns + 1)],
                            start=(c == 0),
                            stop=(c == 3) and gi_sl is None,
                        )
                    if gi_sl is not None:
                        idnt = (
                            ident[:BL, :BL]
                            if t % 2 == 0
                            else ident[BL:128, BL:128]
                        )
                        nc.tensor.matmul(
                            out=ph[:], lhsT=idnt, rhs=gi_sl,
                            start=False, stop=True,
                        )
                r_t = kp.tile([BL, V], BF16, tag="r_t")
                nc.scalar.activation(out=r_t[:], in_=ph_r[:], func=AF.Sigmoid)
                zp = kp.tile([BL, V], BF16, tag="zp")  # z' = 1-z = sigmoid(-a_z)
                nc.scalar.activation(out=zp[:], in_=ph_z[:], func=AF.Sigmoid, scale=-1.0)
                w_t = kp.tile([BL, V], BF16, tag="w_t")  # w = m * z'
                nc.vector.tensor_tensor(
                    out=w_t[:], in0=zp[:],
                    in1=tmask_bf[:, t : t + 1].to_broadcast([BL, V]), op=ALU.mult,
                )
                p_t = kp.tile([BL, V], BF16, tag="p_t")
                if use_biases:
                    ghn_bf = kp.tile([BL, V], BF16, tag="ghn_bf")
                    nc.vector.tensor_tensor(
                        out=ghn_bf[:], in0=ph_n[:], in1=bhn_mat[:], op=ALU.add
                    )
                    nc.vector.tensor_tensor(
                        out=p_t[:], in0=r_t[:], in1=ghn_bf[:], op=ALU.mult
                    )
                else:
                    nc.vector.tensor_tensor(
                        out=p_t[:], in0=ph_n[:], in1=r_t[:], op=ALU.mult
                    )
                gi_n_b0 = gi_n if t % 2 == 0 else gi_odd[t // 2][:]
                a_n = kp.tile([BL, V], BF16, tag="a_n")
                nc.vector.tensor_tensor(out=a_n[:], in0=p_t[:], in1=gi_n_b0, op=ALU.add)
                n_t = kp.tile([BL, V], BF16, tag="n_t")
                nc.scalar.activation(out=n_t[:], in_=a_n[:], func=AF.Tanh)
                u_t = kp.tile([BL, V], BF16, tag="u_t")
                nc.vector.tensor_tensor(out=u_t[:], in0=n_t[:], in1=h_sb[:], op=ALU.subtract)
                t1 = kp.tile([BL, V], BF16, tag="t1")
                nc.vector.tensor_tensor(out=t1[:], in0=w_t[:], in1=u_t[:], op=ALU.mult)
                last_h[0] = nc.vector.tensor_tensor(
                    out=h_sb[:], in0=h_sb[:], in1=t1[:], op=ALU.add
                )
                if t < T - 1:
                    emit_hT(t)

            # interleave gi tiles with early steps so PE stays busy during
            # the gate chains
            emit_gi_tile(0)
            _gi_odd_dma(0)
            if debug:
                nc.gpsimd.dma_start(out=dbg_ext["dbg_gi0"][:], in_=gi_sb[0][:])
                nc.gpsimd.dma_start(out=dbg_ext["dbg_wi0"][:], in_=wi[0][:])
                nc.gpsimd.dma_start(out=dbg_ext["dbg_xt0"][:], in_=xt[0][:])
            emit_step0()
            if debug:
                nc.gpsimd.dma_start(out=dbg_ext["dbg_h0"][:], in_=h_sb[:])
            emit_gi_tile(1)
            _gi_odd_dma(1)
            emit_step(1)
            if debug:
                nc.gpsimd.dma_start(out=dbg_ext["dbg_h1"][:], in_=h_sb[:])
            emit_gi_tile(2)
            _gi_odd_dma(2)
            emit_step(2)
            emit_gi_tile(3)
            _gi_odd_dma(3)
            for t in range(3, T):
                emit_step(t)

            if debug:
                nc.gpsimd.dma_start(out=dbg_ext["dbg_hF"][:], in_=h_sb[:])
            # ======== logits + out ========
            pl = pgp.tile([BL, V], F32, tag="mm")
            nc.tensor.matmul(
                out=pl[:], lhsT=finT0[:], rhs=wo0[:], start=True, stop=False
            )
            nc.tensor.matmul(
                out=pl[:],
                lhsT=finT1[:],
                rhs=wo1[:],
                start=False,
                stop=not use_biases,
            )
            if use_biases:
                nc.tensor.matmul(
                    out=pl[:], lhsT=ones_row[:, :BL], rhs=bo_bf[:], start=False,
                    stop=True,
                )
            out_f = wp.tile([BL, V], F32, tag="out_f")
            nc.vector.tensor_tensor(out=out_f[:], in0=pl[:], in1=h_sb[:], op=ALU.add)
            nc.sync.dma_start(out=out_ext[:], in_=out_f[:])

            # ======== DDI score partial ========
            s_bf = wp.tile([BL, V], BF16, tag="s_bf")
            nc.scalar.activation(out=s_bf[:], in_=out_f[:], func=AF.Sigmoid)
            sT = wp.tile([128, 4 * BL], BF16, tag="sT")
            pts = ptp.tile([128, 4 * BL], BF16, tag="pt", name="pts")
            for c in range(4):
                nc.tensor.transpose(
                    out=pts[:, BL * c : BL * (c + 1)],
                    in_=s_bf[:, 128 * c : 128 * (c + 1)],
                    identity=ident[:BL, :BL],
                )
            nc.vector.tensor_copy(out=sT[:], in_=pts[:])
            pq = pgp.tile([BL, V], F32, tag="mm")
            for c in range(4):
                nc.tensor.matmul(
                    out=pq[:],
                    lhsT=sT[:, BL * c : BL * (c + 1)],
                    rhs=A_bf[:, V * c : V * (c + 1)],
                    start=(c == 0),
                    stop=(c == 3),
                )
            sq = kp.tile([BL, V], F32, tag="sq")
            nc.vector.tensor_tensor(out=sq[:], in0=pq[:], in1=s_bf[:], op=ALU.mult)
            rcol = kp.tile([BL, 1], F32, tag="rcol")
            nc.vector.tensor_reduce(
                out=rcol[:], in_=sq[:], axis=mybir.AxisListType.X, op=ALU.add
            )
            psc = ptp.tile([1, 1], F32, tag="pt")
            nc.tensor.matmul(
                out=psc[:], lhsT=rcol[:], rhs=ones_col[:], start=True, stop=True
            )
            sc_sb = kp.tile([1, 1], F32, tag="sc_sb")
            nc.vector.tensor_copy(out=sc_sb[:], in_=psc[:])
            nc.sync.dma_start(out=ddi_ext[:], in_=sc_sb[:])

    split_waits(nc, limit=1)
    from concourse.library_overlay import lower_extended_insts

    lower_extended_insts(nc)
    return nc


def split_waits(nc, limit=1):
    """walrus in this toolchain only accepts `limit` sem-waits per
    instruction; move excess waits onto same-engine nops placed before."""
    for f in nc.m.functions:
        for bb in f.blocks:
            insts = list(bb.instructions)
            out = []
            for inst in insts:
                si = inst.sync_info
                waits = list(si.on_wait) if si and si.on_wait else []
                if len(waits) > limit:
                    extra, keep = waits[:-limit], waits[-limit:]
                    for w in extra:
                        nop = nc.engines[inst.engine].nop(nofuse=True).ins
                        for f2 in nc.m.functions:
                            for bb2 in f2.blocks:
                                if nop in list(bb2.instructions):
                                    bb2.instructions.remove(nop)
                        nop.sync_info = mybir.SyncInfo(on_wait=[w], on_update=[])
                        out.append(nop)
                    si.on_wait = keep
                out.append(inst)
            bb.instructions[:] = out


def make_in_maps(inputs):
    """Shard the full inputs into per-core input maps."""
    diagnose = np.ascontiguousarray(np.asarray(inputs["diagnose"]).astype(np.int32))
    procedures = np.ascontiguousarray(np.asarray(inputs["procedures"]).astype(np.int32))
    last_meds = np.ascontiguousarray(np.asarray(inputs["last_meds"]).astype(np.int32))

    def wrap16(a):
        """[BL, S] ints -> dma_gather wrapped idx list [16, BL*S/16] i16,
        flat order i = b + BL*s (column-major), wrapped i -> (i%16, i//16)."""
        flat = a.T.reshape(-1).astype(np.int16)
        return np.ascontiguousarray(flat.reshape(-1, 16).T)
    med_hist = np.ascontiguousarray(np.asarray(inputs["med_hist"], np.float32))
    hist_len = np.ascontiguousarray(
        np.asarray(inputs["hist_len"]).astype(np.int32).reshape(B, 1)
    )
    ddi_adj = np.ascontiguousarray(np.asarray(inputs["ddi_adj"], np.float32))
    diag_table = np.ascontiguousarray(np.asarray(inputs["diag_table"], np.float32))
    proc_table = np.ascontiguousarray(np.asarray(inputs["proc_table"], np.float32))
    med_table = np.ascontiguousarray(np.asarray(inputs["med_table"], np.float32))
    W_out = np.asarray(inputs["W_out"], np.float32)
    W_out = np.ascontiguousarray(
        np.concatenate([W_out, np.zeros((V, E), np.float32)], axis=1)
    )
    b_out = np.ascontiguousarray(np.asarray(inputs["b_out"], np.float32).reshape(1, V))
    W_ih = np.ascontiguousarray(np.asarray(inputs["W_ih"], np.float32))
    W_hh = np.ascontiguousarray(np.asarray(inputs["W_hh"], np.float32))
    b_ih = np.ascontiguousarray(np.asarray(inputs["b_ih"], np.float32).reshape(1, G))
    b_hh = np.ascontiguousarray(np.asarray(inputs["b_hh"], np.float32).reshape(1, G))

    in_maps = []
    for k in range(N_CORES):
        sl = slice(k * BL, (k + 1) * BL)
        in_maps.append(
            dict(
                diag_idx16=wrap16(diagnose[sl]),
                proc_idx16=wrap16(procedures[sl]),
                med_idx16=wrap16(last_meds[sl]),
                last_meds=last_meds[sl],
                med_hist=np.ascontiguousarray(med_hist[sl].transpose(1, 0, 2)),
                hist_len=hist_len[sl],
                ddi_adj=ddi_adj,
                diag_table=diag_table,
                proc_table=proc_table,
                med_table=med_table,
                W_out=W_out,
                b_out=b_out,
                W_ih=W_ih,
                W_hh=W_hh,
                b_ih=b_ih,
                b_hh=b_hh,
            )
        )
    use_biases = bool(
        np.any(b_out) or np.any(b_ih) or np.any(b_hh)
    )
    return in_maps, use_biases


_NC_CACHE = {}


def kernel(**inputs):
    from concourse.bass_utils import run_bass_kernel_spmd

    in_maps, use_biases = make_in_maps(inputs)
    if use_biases not in _NC_CACHE:
        _NC_CACHE[use_biases] = build(use_biases)
    nc = _NC_CACHE[use_biases]
    res = run_bass_kernel_spmd(nc, in_maps, core_ids=list(range(N_CORES)))
    out = np.concatenate([res.results[k]["out"] for k in range(N_CORES)], axis=0)
    total = float(sum(float(res.results[k]["ddi"][0, 0]) for k in range(N_CORES)))
    score = np.float32(KGLOSS_SCALE * total / B)
    return out.astype(np.float32), score
